# revision 1
# baseline (speedup 1.0000x reference)
"""LoG (GaussianBlur 3x3 then Laplacian 9x9, reflect-101) as a Bass/Trainium kernel.

Math: both depthwise convs are separable and symmetric, and reflect-101
padding commutes with symmetric-kernel convolution, so the whole pipeline
composes into a single separable 11x11 filter applied to the reflect-101
extension of x:

    out = clip( (A (x) B + B (x) A) * x~  + 1, 0, 255 )

with A = gauss3 conv SMOOTH_9 (11 taps), B = gauss3 conv D2_9 (11 taps).

Implementation per core (batch 4 of the 32 images):
  stage 1 (y-conv, transposed output for free):
      matmul(out=psum[x_chunk, 256], lhsT=x[y_in, x_chunk], rhs=[BandA|BandB|0])
      one fp32r matmul per (x-chunk, y-chunk) computes BOTH u and v columns;
      two y-chunks share one PSUM bank, drained by a single 4D-AP copy.
  stage 2 (x-conv, back to original orientation):
      psum[y_blk, 256] = u_chunk^T @ [BandB|0] + v_chunk^T @ [BandA|BandB(junk)]
      two x-chunks share one PSUM bank; Relu(psum + 1) on ScalarE,
      min(.,255) on GpSimd, DMA out.

fp32r + moving-dim >= 256 runs the PE at 1 cycle/row (vs 4 for fp32).
All fp32r operand tiles are declared float32r so producers round on write
(BIR verifier requirement). Reflect-101 boundaries are folded into the
per-chunk band matrices. x-chunks carry a +-5 halo so stage-2 lhsT reads
stay partition-aligned. Dependency fan-in per matmul is kept on a single
producer engine (walrus has a low per-instruction wait limit).
"""

import numpy as np

N_CORES = 8
BATCH = 32
IMG_PER_CORE = BATCH // N_CORES
H = W = 512
C = 3
RAD = 5  # half width of composed 11-tap filter
NPAD = 0  # unused: exact fp32 matmuls, tight streams


def make_chunks(n):
    step = 103  # 103*5 >= 512, in-size <= 113 <= 128
    bounds = list(range(0, n, step)) + [n]
    out = []
    for s, e in zip(bounds[:-1], bounds[1:]):
        lo, hi = max(s - RAD, 0), min(e + RAD, n)
        out.append((s, e, lo, hi))
    return out

CHUNKS = make_chunks(H)


def make_taps():
    g = np.exp(-((np.arange(3) - 1.0) ** 2) / 2.0)
    g = g / g.sum()
    S = np.array([1, 8, 28, 56, 70, 56, 28, 8, 1], dtype=np.float64)
    D2 = np.array([1, 4, 4, -4, -10, -4, 4, 4, 1], dtype=np.float64)
    return np.convolve(g, S), np.convolve(g, D2)


def make_bands(n):
    """Per chunk: [K, 2*Ni] = [BandA | BandB], reflect-101 folded in."""
    A, B = make_taps()
    bands = []
    for s, e, lo, hi in make_chunks(n):
        ni = e - s
        w = np.zeros((hi - lo, 2 * ni), np.float64)
        for j in range(ni):
            y = s + j
            for t in range(-RAD, RAD + 1):
                src = y + t
                if src < 0:
                    src = -src
                elif src > n - 1:
                    src = 2 * (n - 1) - src
                w[src - lo, j] += A[t + RAD]
                w[src - lo, ni + j] += B[t + RAD]
        bands.append(np.ascontiguousarray(w, np.float32))
    return bands


def _pairs(n):
    ps, i = [], 0
    while i < n:
        ps.append(tuple(range(i, min(i + 2, n))))
        i += 2
    return ps


def build_bass(n_imgs=IMG_PER_CORE, h=H, w=W, c=C):
    import concourse.bacc as bacc
    import concourse.mybir as mybir
    import concourse.tile as tile

    f32 = mybir.dt.float32
    f32r = mybir.dt.float32r
    relu = mybir.ActivationFunctionType.Relu
    chunks = make_chunks(h)
    assert w == h, "chunking shared across axes"
    jpairs = _pairs(len(chunks))

    nc = bacc.Bacc("TRN2", target_bir_lowering=False, debug=False)
    x_d = nc.dram_tensor("x", [n_imgs, h, w, c], f32, kind="ExternalInput")
    out_d = nc.dram_tensor("out", [n_imgs, h, w, c], f32, kind="ExternalOutput")
    band_d = [
        nc.dram_tensor(f"band{i}", [hi - lo, 2 * (e - s)], f32, kind="ExternalInput")
        for i, (s, e, lo, hi) in enumerate(chunks)
    ]

    n_yblk = h // 128

    with tile.TileContext(nc) as tc:
        with (
            tc.tile_pool(name="const", bufs=1) as cpool,
            tc.tile_pool(name="xin", bufs=2) as xpool,
            tc.tile_pool(name="uv", bufs=2) as uvpool,
            tc.tile_pool(name="outp", bufs=2) as opool,
            tc.tile_pool(name="ps", bufs=3, space="PSUM") as pspool,
            tc.tile_pool(name="pso", bufs=3, space="PSUM") as psopool,
        ):
            band = []
            for i, (s, e, lo, hi) in enumerate(chunks):
                tb = cpool.tile([hi - lo, 2 * (e - s)], f32, name=f"band{i}")
                nc.sync.dma_start(tb[:], band_d[i].ap())
                band.append(tb)

            for n in range(n_imgs):
                xrows = []
                for j, (s, e, lo, hi) in enumerate(chunks):
                    xr = xpool.tile([hi - lo, w, c], f32, tag=f"x{j}", name=f"x{j}_{n}")
                    nc.sync.dma_start(xr[:], x_d.ap()[n, lo:hi, :, :])
                    xrows.append(xr)
                outs = []
                for b in range(n_yblk):
                    ot = opool.tile([128, w, c], f32, tag=f"o{b}", name=f"o{b}_{n}")
                    outs.append(ot)
                for ci in range(c):
                    # uv tiles: plane 0 = u, plane 1 = v (transposed: x on partitions)
                    uvts = []
                    for i, (s, e, lo, hi) in enumerate(chunks):
                        uvt = uvpool.tile([hi - lo, 2, h], f32, tag=f"uv{i}", name=f"uv{i}_{n}_{ci}")
                        uvts.append(uvt)
                    # stage 1: y-conv, transposed outputs; 2 tight matmuls per (i,j)
                    for jp in jpairs:
                        nj = chunks[jp[0]][1] - chunks[jp[0]][0]
                        sj0 = chunks[jp[0]][0]
                        seg = 2 * nj
                        for i, (si, ei, loi, hii) in enumerate(chunks):
                            mi = hii - loi
                            ps = pspool.tile([mi, 512], f32, tag="ps")
                            for t, j in enumerate(jp):
                                lhsT = xrows[j][:, loi:hii, ci]
                                nc.tensor.matmul(
                                    ps[:, t * seg : t * seg + nj],
                                    lhsT, band[j][:, 0:nj],
                                    start=True, stop=True,
                                )
                                nc.tensor.matmul(
                                    ps[:, t * seg + nj : t * seg + seg],
                                    lhsT, band[j][:, nj:seg],
                                    start=True, stop=True,
                                )
                            # drain all segments with one 4D-AP copy
                            src = (
                                ps[:, 0 : len(jp) * seg]
                                .rearrange("m (js x) -> m js x", js=len(jp))
                                .rearrange("m js (uv x) -> m uv js x", uv=2)
                            )
                            dst = uvts[i][:, :, sj0 : sj0 + len(jp) * nj].rearrange(
                                "m uv (js x) -> m uv js x", js=len(jp)
                            )
                            if i % 2 == 0:
                                nc.vector.tensor_copy(dst, src)
                            else:
                                nc.scalar.copy(dst, src)
                    # stage 2: x-conv back to original orientation + clip
                    igroups = []
                    cur = []
                    for idx, (s_, e_, lo_, hi_) in enumerate(chunks):
                        ni_ = e_ - s_
                        if cur:
                            n0 = chunks[cur[0]][1] - chunks[cur[0]][0]
                            if n0 != ni_ or (len(cur) + 1) * ni_ > 512:
                                igroups.append(tuple(cur))
                                cur = []
                        cur.append(idx)
                    igroups.append(tuple(cur))
                    for b in range(n_yblk):
                        for ig in igroups:
                            ni = chunks[ig[0]][1] - chunks[ig[0]][0]
                            si0 = chunks[ig[0]][0]
                            pso = psopool.tile([128, 512], f32, tag="pso")
                            for t, i in enumerate(ig):
                                oslice = pso[:, t * ni : t * ni + ni]
                                nc.tensor.matmul(
                                    oslice,
                                    uvts[i][:, 0, b * 128 : (b + 1) * 128],
                                    band[i][:, ni : 2 * ni],
                                    start=True, stop=False,
                                )
                                nc.tensor.matmul(
                                    oslice,
                                    uvts[i][:, 1, b * 128 : (b + 1) * 128],
                                    band[i][:, 0:ni],
                                    start=False, stop=True,
                                )
                            src = pso[:, 0 : len(ig) * ni]
                            dst = outs[b][:, si0 : si0 + len(ig) * ni, ci]
                            nc.scalar.activation(dst, src, relu, bias=1.0)
                            nc.gpsimd.tensor_scalar_min(dst, dst, 255.0)
                for b in range(n_yblk):
                    nc.sync.dma_start(out_d.ap()[n, b * 128 : (b + 1) * 128, :, :], outs[b][:])

    nc.compile()
    return nc


_CACHE = {}


def _get_nc():
    if "nc" not in _CACHE:
        _CACHE["nc"] = build_bass()
    return _CACHE["nc"]


def kernel(x: np.ndarray) -> np.ndarray:
    from concourse import bass_utils

    nc = _get_nc()
    bands = make_bands(H)
    const_map = {f"band{i}": b for i, b in enumerate(bands)}
    x = np.ascontiguousarray(x, dtype=np.float32)
    in_maps = [
        {"x": x[k * IMG_PER_CORE : (k + 1) * IMG_PER_CORE], **const_map}
        for k in range(N_CORES)
    ]
    res = bass_utils.run_bass_kernel_spmd(nc, in_maps, core_ids=list(range(N_CORES)))
    _CACHE["last_result"] = res
    out = np.concatenate([r["out"] for r in res.results], axis=0)
    return out.astype(np.float32)



# revision 16
# speedup vs baseline: 1.2468x; 1.2468x over previous
"""LoG (GaussianBlur 3x3 then Laplacian 9x9, reflect-101) as a Bass/Trainium kernel.

Math: both depthwise convs are separable and symmetric, and reflect-101
padding commutes with symmetric-kernel convolution, so the whole pipeline
composes into a single separable 11x11 filter applied to the reflect-101
extension of x:

    out = clip( A_y(B_x(x)) + B_y(A_x(x)) + 1, 0, 255 )

with A = gauss3 conv SMOOTH_9 (11 taps), B = gauss3 conv D2_9 (11 taps).
Both 1-D convolutions are expressed as dense banded matrices G_A, G_B
([512, 512], reflect-101 folded in), applied via PE matmuls in fp32.
(fp32r would stream 4x faster but rounds operands to ~tf32 -- measured
rel err 0.59 on this problem's +-300k pre-clip intermediates, vs the
2e-2 gate.  fp32's 4 cycles/row is the price of exactness.)

Per core (batch 4 of the 32 images), per channel:

  stage 1 (y-conv, transposed output):
      5 row-chunks (118 output rows + +-5 halo, loaded as full
      128-partition DMA tiles), 4 x-strips of 128.
      One fp32 matmul per (chunk, strip): stationary = x rows, moving =
      [G_A window | G_B window] (236 cols tight) -> one psum slot holds
      BOTH the u and v segments.  Strided-AP copies assemble transposed
      u^T, v^T strip planes in SBUF.

  stage 2 (x-conv, back to original orientation):
      scatter form: for each 128-row output block, a K=1 bf16 zero-fill
      matmul opens the accumulation group covering the whole psum bank
      (hardware per-element has_written would cope with the partially
      overlapping windows, but Tile/CoreSim track pending-zero state at
      bank granularity), then 8 fp32 matmuls (4 aligned x-strips x
      {u,v}) accumulate 133/138-col scatter windows (overlapping by 10).
      Relu(psum + 1) on ScalarE, min(.,255) on GpSimd, DMA out.

Input DMAs use full-128-partition row blocks: the HWDGE descriptor
splitter serializes most of a narrower transfer onto one SDMA engine
(~26 GB/s effective, measured); 128-partition transfers spread across
all 16 engines (~310-370 GB/s).
"""

import numpy as np

N_CORES = 8
BATCH = 32
IMG_PER_CORE = BATCH // N_CORES
H = W = 512
C = 3
RAD = 5  # half width of composed 11-tap filter
TS = 118  # output rows per stage-1 chunk (input rows = TS + 2*RAD <= 128)


def make_chunks():
    """Stage-1 y-chunks: (out_start, out_len, in_lo) with 128 input rows."""
    out = []
    s = 0
    while s < H:
        n = min(TS, H - s)
        lo = min(max(0, s - RAD), H - 128)
        out.append((s, n, lo))
        s += n
    return out


CHUNKS = make_chunks()
NCH = len(CHUNKS)
# psum column slot of each stage-1 chunk's [u | v] segment pair: uniform
# 256-wide slots (so one strided-AP copy drains all four), 236 cols used;
# no slot crosses a 512-fp32 psum bank boundary.  Chunk 4's 40-col u/v
# tails go in bank 2.  ps1 = [128, 1280] = 3 banks.
S1_SLOT = [0, 256, 512, 768]
S1_TAIL_U = 1024  # chunk 4 u -> [1024, 1064)
S1_TAIL_V = 1064  # chunk 4 v -> [1064, 1104)

# stage-2 scatter windows: strip s's 128 x_in rows influence x_out
# [128s - 5, 128s + 133) clipped to [0, 512)
S2_WIN = [(0, 133), (123, 138), (251, 138), (379, 133)]


def make_taps():
    g = np.exp(-((np.arange(3) - 1.0) ** 2) / 2.0)
    g = g / g.sum()
    S = np.array([1, 8, 28, 56, 70, 56, 28, 8, 1], dtype=np.float64)
    D2 = np.array([1, 4, 4, -4, -10, -4, 4, 4, 1], dtype=np.float64)
    return np.convolve(g, S), np.convolve(g, D2)


def scatter_matrix(taps, n=H):
    """G[yi, y] = coefficient of x[yi] in (taps (*) x~)[y], reflect-101."""
    G = np.zeros((n, n), np.float64)
    for y in range(n):
        for t in range(-RAD, RAD + 1):
            src = y + t
            if src < 0:
                src = -src
            elif src > n - 1:
                src = 2 * (n - 1) - src
            G[src, y] += taps[t + RAD]
    return G


def make_bands():
    A, B = make_taps()
    GA = scatter_matrix(A)
    GB = scatter_matrix(B)
    band1 = []  # chunks 0-3: [128, 236] = [GA win 118 | GB win 118]
    for s, n, lo in CHUNKS[:4]:
        w = np.empty((128, 2 * n), np.float64)
        w[:, 0:n] = GA[lo : lo + 128, s : s + n]
        w[:, n : 2 * n] = GB[lo : lo + 128, s : s + n]
        band1.append(np.ascontiguousarray(w, np.float32))
    s, n, lo = CHUNKS[4]
    band1_4u = np.ascontiguousarray(GA[lo : lo + 128, s : s + n], np.float32)
    band1_4v = np.ascontiguousarray(GB[lo : lo + 128, s : s + n], np.float32)
    band2 = []  # per x-strip: [128, 2, w]: plane 0 = GB (for u), 1 = GA (for v)
    for si in range(4):
        x0 = si * 128
        off, wd = S2_WIN[si]
        wb = np.empty((128, 2, wd), np.float64)
        wb[:, 0, :] = GB[x0 : x0 + 128, off : off + wd]
        wb[:, 1, :] = GA[x0 : x0 + 128, off : off + wd]
        for G in (GA, GB):
            assert abs(G[x0 : x0 + 128, :off]).max(initial=0) == 0
            assert abs(G[x0 : x0 + 128, off + wd :]).max(initial=0) == 0
        band2.append(np.ascontiguousarray(wb, np.float32))
    return band1, band1_4u, band1_4v, band2


def build_bass(n_imgs=IMG_PER_CORE, h=H, w=W, c=C):
    import concourse.bacc as bacc
    import concourse.mybir as mybir
    import concourse.tile as tile

    f32 = mybir.dt.float32
    bf16 = mybir.dt.bfloat16
    relu = mybir.ActivationFunctionType.Relu

    nc = bacc.Bacc("TRN2", target_bir_lowering=False, debug=False)
    x_d = nc.dram_tensor("x", [n_imgs, h, w, c], f32, kind="ExternalInput")
    out_d = nc.dram_tensor("out", [n_imgs, h, w, c], f32, kind="ExternalOutput")
    b1_d = [
        nc.dram_tensor(f"band1_{i}", [128, 236], f32, kind="ExternalInput")
        for i in range(4)
    ]
    b1u_d = nc.dram_tensor("band1_4u", [128, 40], f32, kind="ExternalInput")
    b1v_d = nc.dram_tensor("band1_4v", [128, 40], f32, kind="ExternalInput")
    b2_d = [
        nc.dram_tensor(f"band2_{si}", [128, 2, S2_WIN[si][1]], f32, kind="ExternalInput")
        for si in range(4)
    ]

    n_yblk = h // 128
    tail = CHUNKS[4][1]  # 40

    with tile.TileContext(nc) as tc:
        with (
            tc.tile_pool(name="const", bufs=1) as cpool,
            tc.tile_pool(name="xin", bufs=2) as xpool,
            tc.tile_pool(name="uv", bufs=2) as uvpool,
            tc.tile_pool(name="outp", bufs=2) as opool,
            tc.tile_pool(name="ps1", bufs=2, space="PSUM") as ps1pool,
            tc.tile_pool(name="ps2", bufs=2, space="PSUM") as ps2pool,
        ):
            band1 = []
            for i in range(4):
                tb = cpool.tile([128, 236], f32, name=f"band1_{i}")
                nc.sync.dma_start(tb[:], b1_d[i].ap())
                band1.append(tb)
            band1_4u = cpool.tile([128, 40], f32, name="band1_4u")
            nc.sync.dma_start(band1_4u[:], b1u_d.ap())
            band1_4v = cpool.tile([128, 40], f32, name="band1_4v")
            nc.sync.dma_start(band1_4v[:], b1v_d.ap())
            band2 = []
            for si in range(4):
                tb = cpool.tile([128, 2, S2_WIN[si][1]], f32, name=f"band2_{si}")
                nc.sync.dma_start(tb[:], b2_d[si].ap())
                band2.append(tb)
            zs = cpool.tile([1, 128], bf16, name="zstat")
            nc.vector.memset(zs[:], 0.0)
            zm = cpool.tile([1, 512], bf16, name="zmov")
            nc.vector.memset(zm[:], 0.0)

            for n in range(n_imgs):
                xrows = []
                for i, (s, nn_, lo) in enumerate(CHUNKS):
                    xr = xpool.tile([128, w, c], f32, tag=f"x{i}", name=f"x{i}_{n}")
                    nc.sync.dma_start(xr[:], x_d.ap()[n, lo : lo + 128])
                    xrows.append(xr)
                ot = opool.tile([128, n_yblk, w, c], f32, tag="o", name=f"o_{n}")
                for ci in range(c):
                    uts, vts = [], []
                    for si in range(4):
                        ps1 = ps1pool.tile([128, 1280], f32, tag="ps1")
                        xsl = [
                            xrows[i][:, si * 128 : (si + 1) * 128, ci]
                            for i in range(NCH)
                        ]
                        for i in range(4):
                            nc.tensor.matmul(
                                ps1[:, S1_SLOT[i] : S1_SLOT[i] + 236],
                                xsl[i],
                                band1[i][:],
                                start=True,
                                stop=True,
                            )
                        nc.tensor.matmul(
                            ps1[:, S1_TAIL_U : S1_TAIL_U + tail],
                            xsl[4], band1_4u[:], start=True, stop=True,
                        )
                        nc.tensor.matmul(
                            ps1[:, S1_TAIL_V : S1_TAIL_V + tail],
                            xsl[4], band1_4v[:], start=True, stop=True,
                        )
                        ut = uvpool.tile([128, h], f32, tag=f"u{si}", name=f"u{si}_{n}_{ci}")
                        vt = uvpool.tile([128, h], f32, tag=f"v{si}", name=f"v{si}_{n}_{ci}")
                        # chunks 0-3: u at slot+0..118, v at slot+118..236;
                        # chunk 4 u/v are separate 40-col tails in bank 2.
                        slots = ps1[:, 0:1024].rearrange("p (s y) -> p s y", s=4)
                        for dst, poff, toff, cp in (
                            (ut, 0, S1_TAIL_U, nc.vector.tensor_copy),
                            (vt, TS, S1_TAIL_V, nc.scalar.copy),
                        ):
                            cp(
                                dst[:, 0 : 4 * TS].rearrange("p (s x) -> p s x", s=4),
                                slots[:, :, poff : poff + TS],
                            )
                            cp(dst[:, 4 * TS : 4 * TS + tail], ps1[:, toff : toff + tail])
                        uts.append(ut)
                        vts.append(vt)
                    for b in range(n_yblk):
                        ps2 = ps2pool.tile([128, 512], f32, tag="ps2")
                        # K=1 bf16 zero-fill opens the accumulation group over
                        # the full bank; the windowed matmuls then accumulate.
                        nc.tensor.matmul(
                            ps2[:], zs[:], zm[:],
                            start=True, stop=False, skip_group_check=True,
                        )
                        for si in range(4):
                            off, wd = S2_WIN[si]
                            nc.tensor.matmul(
                                ps2[:, off : off + wd],
                                uts[si][:, b * 128 : (b + 1) * 128],
                                band2[si][:, 0],
                                start=False,
                                stop=False,
                                skip_group_check=True,
                            )
                            nc.tensor.matmul(
                                ps2[:, off : off + wd],
                                vts[si][:, b * 128 : (b + 1) * 128],
                                band2[si][:, 1],
                                start=False,
                                stop=(si == 3),
                                skip_group_check=True,
                            )
                        dst = ot[:, b, :, ci]
                        nc.scalar.activation(dst, ps2[:], relu, bias=1.0)
                        nc.gpsimd.tensor_scalar_min(dst, dst, 255.0)
                for b in range(n_yblk):
                    nc.sync.dma_start(
                        out_d.ap()[n, b * 128 : (b + 1) * 128], ot[:, b]
                    )

    nc.compile()
    return nc


_CACHE = {}


def _get_nc():
    if "nc" not in _CACHE:
        _CACHE["nc"] = build_bass()
    return _CACHE["nc"]


def kernel(x: np.ndarray) -> np.ndarray:
    from concourse import bass_utils

    nc = _get_nc()
    band1, band1_4u, band1_4v, band2 = make_bands()
    const_map = {f"band1_{i}": b for i, b in enumerate(band1)}
    const_map["band1_4u"] = band1_4u
    const_map["band1_4v"] = band1_4v
    for si in range(4):
        const_map[f"band2_{si}"] = band2[si]
    x = np.ascontiguousarray(x, dtype=np.float32)
    in_maps = [
        {"x": x[k * IMG_PER_CORE : (k + 1) * IMG_PER_CORE], **const_map}
        for k in range(N_CORES)
    ]
    res = bass_utils.run_bass_kernel_spmd(nc, in_maps, core_ids=list(range(N_CORES)))
    _CACHE["last_result"] = res
    out = np.concatenate([r["out"] for r in res.results], axis=0)
    return out.astype(np.float32)


# revision 19
# speedup vs baseline: 2.0140x; 1.6154x over previous
"""LoG (GaussianBlur 3x3 then Laplacian 9x9, reflect-101) as a Bass/Trainium kernel.

Math: both depthwise convs are separable and symmetric, and reflect-101
padding commutes with symmetric-kernel convolution, so the whole pipeline
composes into a single separable 11x11 filter applied to the reflect-101
extension of x:

    out = clip( A_y(B_x(x)) + B_y(A_x(x)) + 1, 0, 255 )

with A = gauss3 conv SMOOTH_9 (11 taps), B = gauss3 conv D2_9 (11 taps).
Both 1-D convolutions are expressed as dense banded matrices G_A, G_B
([512, 512], reflect-101 folded in), applied via PE matmuls in fp32.
(fp32r would stream 4x faster but rounds operands to ~tf32 -- measured
rel err 0.59 on this problem's +-300k pre-clip intermediates, vs the
2e-2 gate.  fp32's 4 cycles/row is the price of exactness.)

Per core (batch 4 of the 32 images), per channel:

  stage 1 (y-conv, transposed output):
      5 row-chunks (118 output rows + +-5 halo, loaded as full
      128-partition DMA tiles), 4 x-strips of 128.
      One fp32 matmul per (chunk, strip): stationary = x rows, moving =
      [G_A window | G_B window] (236 cols tight) -> one psum slot holds
      BOTH the u and v segments.  Strided-AP copies assemble transposed
      u^T, v^T strip planes in SBUF.

  stage 2 (x-conv, back to original orientation):
      scatter form: for each 128-row output block, a K=1 bf16 zero-fill
      matmul opens the accumulation group covering the whole psum bank
      (hardware per-element has_written would cope with the partially
      overlapping windows, but Tile/CoreSim track pending-zero state at
      bank granularity), then 8 fp32 matmuls (4 aligned x-strips x
      {u,v}) accumulate 133/138-col scatter windows (overlapping by 10).
      Relu(psum + 1) on ScalarE, min(.,255) on GpSimd, DMA out.

Input DMAs use full-128-partition row blocks: the HWDGE descriptor
splitter serializes most of a narrower transfer onto one SDMA engine
(~26 GB/s effective, measured); 128-partition transfers spread across
all 16 engines (~310-370 GB/s).
"""

import numpy as np

N_CORES = 8
BATCH = 32
IMG_PER_CORE = BATCH // N_CORES
H = W = 512
C = 3
RAD = 5  # half width of composed 11-tap filter
TS = 118  # output rows per stage-1 chunk (input rows = TS + 2*RAD <= 128)


def make_chunks():
    """Stage-1 y-chunks: (out_start, out_len, in_lo) with 128 input rows."""
    out = []
    s = 0
    while s < H:
        n = min(TS, H - s)
        lo = min(max(0, s - RAD), H - 128)
        out.append((s, n, lo))
        s += n
    return out


CHUNKS = make_chunks()
NCH = len(CHUNKS)
# psum column slot of each stage-1 chunk's [u | v] segment pair: uniform
# 256-wide slots (so one strided-AP copy drains all four), 236 cols used;
# no slot crosses a 512-fp32 psum bank boundary.  Chunk 4's 40-col u/v
# tails go in bank 2.  ps1 = [128, 1280] = 3 banks.
S1_SLOT = [0, 256, 512, 768]
S1_TAIL_U = 1024  # chunk 4 u -> [1024, 1064)
S1_TAIL_V = 1064  # chunk 4 v -> [1064, 1104)

# stage-2 scatter windows: strip s's 128 x_in rows influence x_out
# [128s - 5, 128s + 133) clipped to [0, 512)
S2_WIN = [(0, 133), (123, 138), (251, 138), (379, 133)]


def make_taps():
    g = np.exp(-((np.arange(3) - 1.0) ** 2) / 2.0)
    g = g / g.sum()
    S = np.array([1, 8, 28, 56, 70, 56, 28, 8, 1], dtype=np.float64)
    D2 = np.array([1, 4, 4, -4, -10, -4, 4, 4, 1], dtype=np.float64)
    return np.convolve(g, S), np.convolve(g, D2)


def scatter_matrix(taps, n=H):
    """G[yi, y] = coefficient of x[yi] in (taps (*) x~)[y], reflect-101."""
    G = np.zeros((n, n), np.float64)
    for y in range(n):
        for t in range(-RAD, RAD + 1):
            src = y + t
            if src < 0:
                src = -src
            elif src > n - 1:
                src = 2 * (n - 1) - src
            G[src, y] += taps[t + RAD]
    return G


def make_bands():
    A, B = make_taps()
    GA = scatter_matrix(A)
    GB = scatter_matrix(B)
    band1 = []  # chunks 0-3: [128, 236] = [GA win 118 | GB win 118]
    for s, n, lo in CHUNKS[:4]:
        w = np.empty((128, 2 * n), np.float64)
        w[:, 0:n] = GA[lo : lo + 128, s : s + n]
        w[:, n : 2 * n] = GB[lo : lo + 128, s : s + n]
        band1.append(np.ascontiguousarray(w, np.float32))
    s, n, lo = CHUNKS[4]
    band1_4u = np.ascontiguousarray(GA[lo : lo + 128, s : s + n], np.float32)
    band1_4v = np.ascontiguousarray(GB[lo : lo + 128, s : s + n], np.float32)
    band2 = []  # per x-strip: [128, 2, w]: plane 0 = GB (for u), 1 = GA (for v)
    for si in range(4):
        x0 = si * 128
        off, wd = S2_WIN[si]
        wb = np.empty((128, 2, wd), np.float64)
        wb[:, 0, :] = GB[x0 : x0 + 128, off : off + wd]
        wb[:, 1, :] = GA[x0 : x0 + 128, off : off + wd]
        for G in (GA, GB):
            assert abs(G[x0 : x0 + 128, :off]).max(initial=0) == 0
            assert abs(G[x0 : x0 + 128, off + wd :]).max(initial=0) == 0
        band2.append(np.ascontiguousarray(wb, np.float32))
    return band1, band1_4u, band1_4v, band2


def build_bass(n_imgs=IMG_PER_CORE, h=H, w=W, c=C):
    import concourse.bacc as bacc
    import concourse.mybir as mybir
    import concourse.tile as tile

    f32 = mybir.dt.float32
    bf16 = mybir.dt.bfloat16
    copyf = mybir.ActivationFunctionType.Copy

    nc = bacc.Bacc("TRN2", target_bir_lowering=False, debug=False)
    x_d = nc.dram_tensor("x", [n_imgs, h, w, c], f32, kind="ExternalInput")
    out_d = nc.dram_tensor("out", [n_imgs, h, w, c], f32, kind="ExternalOutput")
    b1_d = [
        nc.dram_tensor(f"band1_{i}", [128, 236], f32, kind="ExternalInput")
        for i in range(4)
    ]
    b1u_d = nc.dram_tensor("band1_4u", [128, 40], f32, kind="ExternalInput")
    b1v_d = nc.dram_tensor("band1_4v", [128, 40], f32, kind="ExternalInput")
    b2_d = [
        nc.dram_tensor(f"band2_{si}", [128, 2, S2_WIN[si][1]], f32, kind="ExternalInput")
        for si in range(4)
    ]

    n_yblk = h // 128
    tail = CHUNKS[4][1]  # 40

    with tile.TileContext(nc) as tc:
        with (
            tc.tile_pool(name="const", bufs=1) as cpool,
            tc.tile_pool(name="xin", bufs=2) as xpool,
            tc.tile_pool(name="uv", bufs=2) as uvpool,
            tc.tile_pool(name="outp", bufs=2) as opool,
            tc.tile_pool(name="clip", bufs=3) as clpool,
            tc.tile_pool(name="ps1", bufs=2, space="PSUM") as ps1pool,
            tc.tile_pool(name="ps2", bufs=2, space="PSUM") as ps2pool,
        ):
            band1 = []
            for i in range(4):
                tb = cpool.tile([128, 236], f32, name=f"band1_{i}")
                nc.sync.dma_start(tb[:], b1_d[i].ap())
                band1.append(tb)
            band1_4u = cpool.tile([128, 40], f32, name="band1_4u")
            nc.sync.dma_start(band1_4u[:], b1u_d.ap())
            band1_4v = cpool.tile([128, 40], f32, name="band1_4v")
            nc.sync.dma_start(band1_4v[:], b1v_d.ap())
            band2 = []
            for si in range(4):
                tb = cpool.tile([128, 2, S2_WIN[si][1]], f32, name=f"band2_{si}")
                nc.sync.dma_start(tb[:], b2_d[si].ap())
                band2.append(tb)
            zs = cpool.tile([1, 128], bf16, name="zstat")
            nc.vector.memset(zs[:], 0.0)
            zm = cpool.tile([1, 512], bf16, name="zmov")
            nc.vector.memset(zm[:], 0.0)

            for n in range(n_imgs):
                xrows = []
                for i, (s, nn_, lo) in enumerate(CHUNKS):
                    xr = xpool.tile([128, w, c], f32, tag=f"x{i}", name=f"x{i}_{n}")
                    nc.sync.dma_start(xr[:], x_d.ap()[n, lo : lo + 128])
                    xrows.append(xr)
                ot = opool.tile([128, n_yblk, w, c], f32, tag="o", name=f"o_{n}")
                for ci in range(c):
                    uts, vts = [], []
                    for si in range(4):
                        ps1 = ps1pool.tile([128, 1280], f32, tag="ps1")
                        xsl = [
                            xrows[i][:, si * 128 : (si + 1) * 128, ci]
                            for i in range(NCH)
                        ]
                        for i in range(4):
                            nc.tensor.matmul(
                                ps1[:, S1_SLOT[i] : S1_SLOT[i] + 236],
                                xsl[i],
                                band1[i][:],
                                start=True,
                                stop=True,
                            )
                        nc.tensor.matmul(
                            ps1[:, S1_TAIL_U : S1_TAIL_U + tail],
                            xsl[4], band1_4u[:], start=True, stop=True,
                        )
                        nc.tensor.matmul(
                            ps1[:, S1_TAIL_V : S1_TAIL_V + tail],
                            xsl[4], band1_4v[:], start=True, stop=True,
                        )
                        ut = uvpool.tile([128, h], f32, tag=f"u{si}", name=f"u{si}_{n}_{ci}")
                        vt = uvpool.tile([128, h], f32, tag=f"v{si}", name=f"v{si}_{n}_{ci}")
                        # chunks 0-3: u at slot+0..118, v at slot+118..236;
                        # chunk 4 u/v are separate 40-col tails in bank 2.
                        slots = ps1[:, 0:1024].rearrange("p (s y) -> p s y", s=4)
                        for dst, poff, toff, cp in (
                            (ut, 0, S1_TAIL_U, nc.vector.tensor_copy),
                            (vt, TS, S1_TAIL_V, nc.scalar.copy),
                        ):
                            cp(
                                dst[:, 0 : 4 * TS].rearrange("p (s x) -> p s x", s=4),
                                slots[:, :, poff : poff + TS],
                            )
                            cp(dst[:, 4 * TS : 4 * TS + tail], ps1[:, toff : toff + tail])
                        uts.append(ut)
                        vts.append(vt)
                    for b in range(n_yblk):
                        ps2 = ps2pool.tile([128, 512], f32, tag="ps2")
                        # K=1 bf16 zero-fill opens the accumulation group over
                        # the full bank; the windowed matmuls then accumulate.
                        nc.tensor.matmul(
                            ps2[:], zs[:], zm[:],
                            start=True, stop=False, skip_group_check=True,
                        )
                        for si in range(4):
                            off, wd = S2_WIN[si]
                            nc.tensor.matmul(
                                ps2[:, off : off + wd],
                                uts[si][:, b * 128 : (b + 1) * 128],
                                band2[si][:, 0],
                                start=False,
                                stop=False,
                                skip_group_check=True,
                            )
                            nc.tensor.matmul(
                                ps2[:, off : off + wd],
                                vts[si][:, b * 128 : (b + 1) * 128],
                                band2[si][:, 1],
                                start=False,
                                stop=(si == 3),
                                skip_group_check=True,
                            )
                        # clip(lap+1, 0, 255) = clamp(lap, -1, 254) + 1:
                        # DVE clamps psum into a contiguous tmp (GpSimd was
                        # ~7us per strided min op), ACT adds 1 into the
                        # channel-strided output.
                        tmp = clpool.tile([128, 512], f32, tag="cl")
                        nc.vector.tensor_scalar(
                            tmp[:], ps2[:], 254.0, -1.0,
                            mybir.AluOpType.min, mybir.AluOpType.max,
                        )
                        nc.scalar.activation(
                            ot[:, b, :, ci], tmp[:], copyf, bias=1.0
                        )
                for b in range(n_yblk):
                    nc.sync.dma_start(
                        out_d.ap()[n, b * 128 : (b + 1) * 128], ot[:, b]
                    )

    nc.compile()
    return nc


_CACHE = {}


def _get_nc():
    if "nc" not in _CACHE:
        _CACHE["nc"] = build_bass()
    return _CACHE["nc"]


def kernel(x: np.ndarray) -> np.ndarray:
    from concourse import bass_utils

    nc = _get_nc()
    band1, band1_4u, band1_4v, band2 = make_bands()
    const_map = {f"band1_{i}": b for i, b in enumerate(band1)}
    const_map["band1_4u"] = band1_4u
    const_map["band1_4v"] = band1_4v
    for si in range(4):
        const_map[f"band2_{si}"] = band2[si]
    x = np.ascontiguousarray(x, dtype=np.float32)
    in_maps = [
        {"x": x[k * IMG_PER_CORE : (k + 1) * IMG_PER_CORE], **const_map}
        for k in range(N_CORES)
    ]
    res = bass_utils.run_bass_kernel_spmd(nc, in_maps, core_ids=list(range(N_CORES)))
    _CACHE["last_result"] = res
    out = np.concatenate([r["out"] for r in res.results], axis=0)
    return out.astype(np.float32)


# revision 20
# speedup vs baseline: 2.0207x; 1.0033x over previous
"""LoG as fp16 hi/lo-pair matmuls (1 cycle/row vs fp32's 4).

Same two-stage banded-matrix structure as kernel.py, but every operand is
represented as an fp16 pair (hi + lo ~ 21 mantissa bits, enough for the
+-300k pre-clip intermediates at the 2e-2 gate):

  x  = x_hi + x_lo           (split on host; fp16 DMA halves input bytes
                              per tensor, so total input bytes match fp32)
  G  = G_hi + G_lo           (band constants, split on host)
  u  = u_hi + u_lo           (split on-chip during the stage-1 drain:
                              u_hi = fp16(psum) via ScalarE round-on-write,
                              u_lo = psum - u_hi via VectorE)

Products kept: hi*hi + hi*lo + lo*hi (lo*lo ~ 2^-22 relative, dropped).
All three matmuls of a term target the SAME psum columns and accumulate
(start only on the first matmul of the bank's group).

Stage-2's first matmul (strip 0, u_hi x G_B-hi) streams the full 512-col
row so the accumulation group's opening matmul covers the whole psum bank
(Tile/CoreSim track pending-zero state at bank granularity).
"""

import numpy as np

N_CORES = 8
BATCH = 32
IMG_PER_CORE = BATCH // N_CORES
H = W = 512
C = 3
RAD = 5
TS = 118


def make_chunks():
    out = []
    s = 0
    while s < H:
        n = min(TS, H - s)
        lo = min(max(0, s - RAD), H - 128)
        out.append((s, n, lo))
        s += n
    return out


CHUNKS = make_chunks()
NCH = len(CHUNKS)
S1_SLOT = [0, 256, 512, 768]
S1_TAIL_U = 1024
S1_TAIL_V = 1064
S2_WIN = [(0, 133), (123, 138), (251, 138), (379, 133)]


def make_taps():
    g = np.exp(-((np.arange(3) - 1.0) ** 2) / 2.0)
    g = g / g.sum()
    S = np.array([1, 8, 28, 56, 70, 56, 28, 8, 1], dtype=np.float64)
    D2 = np.array([1, 4, 4, -4, -10, -4, 4, 4, 1], dtype=np.float64)
    return np.convolve(g, S), np.convolve(g, D2)


def scatter_matrix(taps, n=H):
    G = np.zeros((n, n), np.float64)
    for y in range(n):
        for t in range(-RAD, RAD + 1):
            src = y + t
            if src < 0:
                src = -src
            elif src > n - 1:
                src = 2 * (n - 1) - src
            G[src, y] += taps[t + RAD]
    return G


def _pair(a):
    hi = a.astype(np.float16)
    lo = (a - hi.astype(np.float64)).astype(np.float16)
    return hi, lo


def make_bands():
    A, B = make_taps()
    GA = scatter_matrix(A)
    GB = scatter_matrix(B)
    out = {}
    # stage 1: per chunk 0-3: [A win | B win] hi and lo as separate tensors
    for i, (s, n, lo_) in enumerate(CHUNKS[:4]):
        w = np.empty((128, 2 * n), np.float64)
        w[:, 0:n] = GA[lo_ : lo_ + 128, s : s + n]
        w[:, n : 2 * n] = GB[lo_ : lo_ + 128, s : s + n]
        hi, lo = _pair(w)
        out[f"band1h_{i}"] = hi
        out[f"band1l_{i}"] = lo
    s, n, lo_ = CHUNKS[4]
    w = np.empty((128, 2 * n), np.float64)
    w[:, 0:n] = GA[lo_ : lo_ + 128, s : s + n]
    w[:, n : 2 * n] = GB[lo_ : lo_ + 128, s : s + n]
    hi, lo = _pair(w)
    out["band1h_4"] = hi
    out["band1l_4"] = lo
    # stage 2: per strip: [128, 2, w] (plane 0 = GB for u, 1 = GA for v)
    for si in range(4):
        x0 = si * 128
        off, wd = S2_WIN[si]
        wb = np.empty((128, 2, wd), np.float64)
        wb[:, 0, :] = GB[x0 : x0 + 128, off : off + wd]
        wb[:, 1, :] = GA[x0 : x0 + 128, off : off + wd]
        for G in (GA, GB):
            assert abs(G[x0 : x0 + 128, :off]).max(initial=0) == 0
            assert abs(G[x0 : x0 + 128, off + wd :]).max(initial=0) == 0
        hi, lo = _pair(wb)
        out[f"band2h_{si}"] = hi
        out[f"band2l_{si}"] = lo
    # strip 0's u-hi matmul streams the full 512-col row (group opener)
    w = np.zeros((128, 512), np.float64)
    w[:, 0:133] = GB[0:128, 0:133]
    out["band2h_0u_full"] = _pair(w)[0]
    return out


def build_bass(n_imgs=IMG_PER_CORE, h=H, w=W, c=C):
    import concourse.bacc as bacc
    import concourse.mybir as mybir
    import concourse.tile as tile

    f32 = mybir.dt.float32
    f16 = mybir.dt.float16
    copyf = mybir.ActivationFunctionType.Copy

    nc = bacc.Bacc("TRN2", target_bir_lowering=False, debug=False)
    xh_d = nc.dram_tensor("xh", [n_imgs, h, w, c], f16, kind="ExternalInput")
    xl_d = nc.dram_tensor("xl", [n_imgs, h, w, c], f16, kind="ExternalInput")
    out_d = nc.dram_tensor("out", [n_imgs, h, w, c], f32, kind="ExternalOutput")
    bands = make_bands()
    band_d = {
        k: nc.dram_tensor(k, list(v.shape), f16, kind="ExternalInput")
        for k, v in bands.items()
    }

    n_yblk = h // 128
    tail = CHUNKS[4][1]  # 40

    with tile.TileContext(nc) as tc:
        with (
            tc.tile_pool(name="const", bufs=1) as cpool,
            tc.tile_pool(name="xin", bufs=2) as xpool,
            tc.tile_pool(name="uv", bufs=2) as uvpool,
            tc.tile_pool(name="outp", bufs=2) as opool,
            tc.tile_pool(name="clip", bufs=3) as clpool,
            tc.tile_pool(name="ps1", bufs=2, space="PSUM") as ps1pool,
            tc.tile_pool(name="ps2", bufs=2, space="PSUM") as ps2pool,
        ):
            bt = {}
            for k, v in bands.items():
                t = cpool.tile(list(v.shape), f16, name=k)
                nc.sync.dma_start(t[:], band_d[k].ap())
                bt[k] = t

            for n in range(n_imgs):
                xhr, xlr = [], []
                for i, (s, nn_, lo) in enumerate(CHUNKS):
                    th = xpool.tile([128, w, c], f16, tag=f"xh{i}", name=f"xh{i}_{n}")
                    nc.sync.dma_start(th[:], xh_d.ap()[n, lo : lo + 128])
                    xhr.append(th)
                    tl = xpool.tile([128, w, c], f16, tag=f"xl{i}", name=f"xl{i}_{n}")
                    nc.sync.dma_start(tl[:], xl_d.ap()[n, lo : lo + 128])
                    xlr.append(tl)
                ot = opool.tile([128, n_yblk, w, c], f32, tag="o", name=f"o_{n}")
                for ci in range(c):
                    uts, vts = [], []
                    for si in range(4):
                        ps1 = ps1pool.tile([128, 1280], f32, tag="ps1")
                        for i in range(NCH):
                            xh = xhr[i][:, si * 128 : (si + 1) * 128, ci]
                            xl = xlr[i][:, si * 128 : (si + 1) * 128, ci]
                            wd = 2 * CHUNKS[i][1]
                            if i < 4:
                                dst = ps1[:, S1_SLOT[i] : S1_SLOT[i] + wd]
                            else:
                                dst = ps1[:, S1_TAIL_U : S1_TAIL_U + wd]
                            nc.tensor.matmul(
                                dst, xh, bt[f"band1h_{i}"][:],
                                start=True, stop=False, skip_group_check=True,
                            )
                            nc.tensor.matmul(
                                dst, xh, bt[f"band1l_{i}"][:],
                                start=False, stop=False, skip_group_check=True,
                            )
                            nc.tensor.matmul(
                                dst, xl, bt[f"band1h_{i}"][:],
                                start=False, stop=True, skip_group_check=True,
                            )
                        # drains: u_hi = fp16(psum) on ScalarE (round on
                        # write), u_lo = psum - u_hi on VectorE.
                        uh = uvpool.tile([128, h], f16, tag=f"uh{si}", name=f"uh{si}_{n}_{ci}")
                        ul = uvpool.tile([128, h], f16, tag=f"ul{si}", name=f"ul{si}_{n}_{ci}")
                        vh = uvpool.tile([128, h], f16, tag=f"vh{si}", name=f"vh{si}_{n}_{ci}")
                        vl = uvpool.tile([128, h], f16, tag=f"vl{si}", name=f"vl{si}_{n}_{ci}")
                        slots = ps1[:, 0:1024].rearrange("p (s y) -> p s y", s=4)
                        for hi_t, lo_t, poff, toff in (
                            (uh, ul, 0, S1_TAIL_U),
                            (vh, vl, TS, S1_TAIL_V),
                        ):
                            src = slots[:, :, poff : poff + TS]
                            hi4 = hi_t[:, 0 : 4 * TS].rearrange("p (s x) -> p s x", s=4)
                            nc.scalar.copy(hi4, src)
                            nc.vector.tensor_tensor(
                                lo_t[:, 0 : 4 * TS].rearrange("p (s x) -> p s x", s=4),
                                src, hi4, mybir.AluOpType.subtract,
                            )
                            tsrc = ps1[:, toff : toff + tail]
                            nc.scalar.copy(hi_t[:, 4 * TS : 4 * TS + tail], tsrc)
                            nc.vector.tensor_tensor(
                                lo_t[:, 4 * TS : 4 * TS + tail],
                                tsrc, hi_t[:, 4 * TS : 4 * TS + tail],
                                mybir.AluOpType.subtract,
                            )
                        uts.append((uh, ul))
                        vts.append((vh, vl))
                    for b in range(n_yblk):
                        ps2 = ps2pool.tile([128, 512], f32, tag="ps2")
                        for si in range(4):
                            off, wd = S2_WIN[si]
                            win = ps2[:, off : off + wd]
                            ysl = slice(b * 128, (b + 1) * 128)
                            uh, ul = uts[si]
                            vh, vl = vts[si]
                            if si == 0:
                                nc.tensor.matmul(
                                    ps2[:], uh[:, ysl], bt["band2h_0u_full"][:],
                                    start=True, stop=False, skip_group_check=True,
                                )
                            else:
                                nc.tensor.matmul(
                                    win, uh[:, ysl], bt[f"band2h_{si}"][:, 0],
                                    start=False, stop=False, skip_group_check=True,
                                )
                            nc.tensor.matmul(
                                win, uh[:, ysl], bt[f"band2l_{si}"][:, 0],
                                start=False, stop=False, skip_group_check=True,
                            )
                            nc.tensor.matmul(
                                win, ul[:, ysl], bt[f"band2h_{si}"][:, 0],
                                start=False, stop=False, skip_group_check=True,
                            )
                            nc.tensor.matmul(
                                win, vh[:, ysl], bt[f"band2h_{si}"][:, 1],
                                start=False, stop=False, skip_group_check=True,
                            )
                            nc.tensor.matmul(
                                win, vh[:, ysl], bt[f"band2l_{si}"][:, 1],
                                start=False, stop=False, skip_group_check=True,
                            )
                            nc.tensor.matmul(
                                win, vl[:, ysl], bt[f"band2h_{si}"][:, 1],
                                start=False, stop=(si == 3), skip_group_check=True,
                            )
                        tmp = clpool.tile([128, 512], f32, tag="cl")
                        nc.vector.tensor_scalar(
                            tmp[:], ps2[:], 254.0, -1.0,
                            mybir.AluOpType.min, mybir.AluOpType.max,
                        )
                        nc.scalar.activation(
                            ot[:, b, :, ci], tmp[:], copyf, bias=1.0
                        )
                for b in range(n_yblk):
                    nc.sync.dma_start(
                        out_d.ap()[n, b * 128 : (b + 1) * 128], ot[:, b]
                    )

    nc.compile()
    return nc


_CACHE = {}


def _get_nc():
    if "nc" not in _CACHE:
        _CACHE["nc"] = build_bass()
    return _CACHE["nc"]


def kernel(x: np.ndarray) -> np.ndarray:
    from concourse import bass_utils

    nc = _get_nc()
    x = np.ascontiguousarray(x, dtype=np.float32)
    xh = x.astype(np.float16)
    xl = (x - xh.astype(np.float32)).astype(np.float16)
    const_map = dict(make_bands())
    in_maps = [
        {
            "xh": xh[k * IMG_PER_CORE : (k + 1) * IMG_PER_CORE],
            "xl": xl[k * IMG_PER_CORE : (k + 1) * IMG_PER_CORE],
            **const_map,
        }
        for k in range(N_CORES)
    ]
    res = bass_utils.run_bass_kernel_spmd(nc, in_maps, core_ids=list(range(N_CORES)))
    _CACHE["last_result"] = res
    out = np.concatenate([r["out"] for r in res.results], axis=0)
    return out.astype(np.float32)


# revision 22
# speedup vs baseline: 2.2251x; 1.1011x over previous
"""LoG as fp16 hi/lo-pair matmuls (1 cycle/row vs fp32's 4).

Two-stage banded-matrix structure; every operand is an fp16 pair
(hi + lo ~ 21 mantissa bits, enough for the +-300k pre-clip
intermediates at the 2e-2 gate):

  x  = x_hi + x_lo           (split on host; fp16 DMA halves input bytes
                              per tensor, so total input bytes match fp32)
  G  = G_hi + G_lo           (band constants, split on host)
  u  = u_hi + u_lo           (split on-chip during the stage-1 drain:
                              u_hi = fp16(psum) via ScalarE round-on-write,
                              u_lo = psum - u_hi via VectorE)

Products kept: hi*hi + hi*lo + lo*hi (lo*lo ~ 2^-22 relative, dropped).
All three matmuls of a term target the SAME psum columns and accumulate
(start only on the first matmul of the bank's group).

Stage-2's first matmul (strip 0, u_hi x G_B-hi) streams the full 512-col
row so the accumulation group's opening matmul covers the whole psum bank
(Tile/CoreSim track pending-zero state at bank granularity).

The channel loop is software-pipelined one step: stage 2 of job k runs
after stage 1 of job k+1, so the PE fills the ~1.8 us drain-latency
bubble at each stage-1 -> stage-2 transition with independent stage-1
matmuls instead of head-of-line blocking on the last strip's drains.

Input DMAs use full-128-partition row blocks: the HWDGE descriptor
splitter serializes most of a narrower transfer onto one SDMA engine;
128-partition transfers spread across all 16 engines.
"""

import numpy as np

N_CORES = 8
BATCH = 32
IMG_PER_CORE = BATCH // N_CORES
H = W = 512
C = 3
RAD = 5
TS = 118


def make_chunks():
    out = []
    s = 0
    while s < H:
        n = min(TS, H - s)
        lo = min(max(0, s - RAD), H - 128)
        out.append((s, n, lo))
        s += n
    return out


CHUNKS = make_chunks()
NCH = len(CHUNKS)
S1_SLOT = [0, 256, 512, 768]
S1_TAIL_U = 1024
S1_TAIL_V = 1064
S2_WIN = [(0, 133), (123, 138), (251, 138), (379, 133)]


def make_taps():
    g = np.exp(-((np.arange(3) - 1.0) ** 2) / 2.0)
    g = g / g.sum()
    S = np.array([1, 8, 28, 56, 70, 56, 28, 8, 1], dtype=np.float64)
    D2 = np.array([1, 4, 4, -4, -10, -4, 4, 4, 1], dtype=np.float64)
    return np.convolve(g, S), np.convolve(g, D2)


def scatter_matrix(taps, n=H):
    G = np.zeros((n, n), np.float64)
    for y in range(n):
        for t in range(-RAD, RAD + 1):
            src = y + t
            if src < 0:
                src = -src
            elif src > n - 1:
                src = 2 * (n - 1) - src
            G[src, y] += taps[t + RAD]
    return G


def _pair(a):
    hi = a.astype(np.float16)
    lo = (a - hi.astype(np.float64)).astype(np.float16)
    return hi, lo


def make_bands():
    A, B = make_taps()
    GA = scatter_matrix(A)
    GB = scatter_matrix(B)
    out = {}
    # stage 1: per chunk: [A win | B win] hi and lo as separate tensors
    for i, (s, n, lo_) in enumerate(CHUNKS):
        w = np.empty((128, 2 * n), np.float64)
        w[:, 0:n] = GA[lo_ : lo_ + 128, s : s + n]
        w[:, n : 2 * n] = GB[lo_ : lo_ + 128, s : s + n]
        hi, lo = _pair(w)
        out[f"band1h_{i}"] = hi
        out[f"band1l_{i}"] = lo
    # stage 2: per strip: [128, 2, w] (plane 0 = GB for u, 1 = GA for v)
    for si in range(4):
        x0 = si * 128
        off, wd = S2_WIN[si]
        wb = np.empty((128, 2, wd), np.float64)
        wb[:, 0, :] = GB[x0 : x0 + 128, off : off + wd]
        wb[:, 1, :] = GA[x0 : x0 + 128, off : off + wd]
        for G in (GA, GB):
            assert abs(G[x0 : x0 + 128, :off]).max(initial=0) == 0
            assert abs(G[x0 : x0 + 128, off + wd :]).max(initial=0) == 0
        hi, lo = _pair(wb)
        out[f"band2h_{si}"] = hi
        out[f"band2l_{si}"] = lo
    # strip 0's u-hi matmul streams the full 512-col row (group opener)
    w = np.zeros((128, 512), np.float64)
    w[:, 0:133] = GB[0:128, 0:133]
    out["band2h_0u_full"] = _pair(w)[0]
    return out


def build_bass(n_imgs=IMG_PER_CORE, h=H, w=W, c=C):
    import concourse.bacc as bacc
    import concourse.mybir as mybir
    import concourse.tile as tile

    f32 = mybir.dt.float32
    f16 = mybir.dt.float16
    copyf = mybir.ActivationFunctionType.Copy

    nc = bacc.Bacc("TRN2", target_bir_lowering=False, debug=False)
    xh_d = nc.dram_tensor("xh", [n_imgs, h, w, c], f16, kind="ExternalInput")
    xl_d = nc.dram_tensor("xl", [n_imgs, h, w, c], f16, kind="ExternalInput")
    out_d = nc.dram_tensor("out", [n_imgs, h, w, c], f32, kind="ExternalOutput")
    bands = make_bands()
    band_d = {
        k: nc.dram_tensor(k, list(v.shape), f16, kind="ExternalInput")
        for k, v in bands.items()
    }

    n_yblk = h // 128
    tail = CHUNKS[4][1]  # 40

    with tile.TileContext(nc) as tc:
        with (
            tc.tile_pool(name="const", bufs=1) as cpool,
            tc.tile_pool(name="xin", bufs=2) as xpool,
            tc.tile_pool(name="uv", bufs=2) as uvpool,
            tc.tile_pool(name="outp", bufs=2) as opool,
            tc.tile_pool(name="clip", bufs=3) as clpool,
            tc.tile_pool(name="ps1", bufs=2, space="PSUM") as ps1pool,
            tc.tile_pool(name="ps2", bufs=2, space="PSUM") as ps2pool,
        ):
            bt = {}
            for k, v in bands.items():
                t = cpool.tile(list(v.shape), f16, name=k)
                nc.sync.dma_start(t[:], band_d[k].ap())
                bt[k] = t

            def load_x(n):
                xhr, xlr = [], []
                for i, (s, nn_, lo) in enumerate(CHUNKS):
                    th = xpool.tile([128, w, c], f16, tag=f"xh{i}", name=f"xh{i}_{n}")
                    nc.sync.dma_start(th[:], xh_d.ap()[n, lo : lo + 128])
                    xhr.append(th)
                    tl = xpool.tile([128, w, c], f16, tag=f"xl{i}", name=f"xl{i}_{n}")
                    nc.sync.dma_start(tl[:], xl_d.ap()[n, lo : lo + 128])
                    xlr.append(tl)
                return xhr, xlr

            def stage1(n, ci, xhr, xlr):
                uts, vts = [], []
                for si in range(4):
                    ps1 = ps1pool.tile([128, 1280], f32, tag="ps1")
                    for i in range(NCH):
                        xh = xhr[i][:, si * 128 : (si + 1) * 128, ci]
                        xl = xlr[i][:, si * 128 : (si + 1) * 128, ci]
                        wd = 2 * CHUNKS[i][1]
                        if i < 4:
                            dst = ps1[:, S1_SLOT[i] : S1_SLOT[i] + wd]
                        else:
                            dst = ps1[:, S1_TAIL_U : S1_TAIL_U + wd]
                        nc.tensor.matmul(
                            dst, xh, bt[f"band1h_{i}"][:],
                            start=True, stop=False, skip_group_check=True,
                        )
                        nc.tensor.matmul(
                            dst, xh, bt[f"band1l_{i}"][:],
                            start=False, stop=False, skip_group_check=True,
                        )
                        nc.tensor.matmul(
                            dst, xl, bt[f"band1h_{i}"][:],
                            start=False, stop=True, skip_group_check=True,
                        )
                    # drains: u_hi = fp16(psum) on ScalarE (round on write),
                    # u_lo = psum - u_hi on VectorE.
                    uh = uvpool.tile([128, h], f16, tag=f"uh{si}", name=f"uh{si}_{n}_{ci}")
                    ul = uvpool.tile([128, h], f16, tag=f"ul{si}", name=f"ul{si}_{n}_{ci}")
                    vh = uvpool.tile([128, h], f16, tag=f"vh{si}", name=f"vh{si}_{n}_{ci}")
                    vl = uvpool.tile([128, h], f16, tag=f"vl{si}", name=f"vl{si}_{n}_{ci}")
                    slots = ps1[:, 0:1024].rearrange("p (s y) -> p s y", s=4)
                    for hi_t, lo_t, poff, toff in (
                        (uh, ul, 0, S1_TAIL_U),
                        (vh, vl, TS, S1_TAIL_V),
                    ):
                        src = slots[:, :, poff : poff + TS]
                        hi4 = hi_t[:, 0 : 4 * TS].rearrange("p (s x) -> p s x", s=4)
                        nc.scalar.copy(hi4, src)
                        nc.vector.tensor_tensor(
                            lo_t[:, 0 : 4 * TS].rearrange("p (s x) -> p s x", s=4),
                            src, hi4, mybir.AluOpType.subtract,
                        )
                        tsrc = ps1[:, toff : toff + tail]
                        nc.scalar.copy(hi_t[:, 4 * TS : 4 * TS + tail], tsrc)
                        nc.vector.tensor_tensor(
                            lo_t[:, 4 * TS : 4 * TS + tail],
                            tsrc, hi_t[:, 4 * TS : 4 * TS + tail],
                            mybir.AluOpType.subtract,
                        )
                    uts.append((uh, ul))
                    vts.append((vh, vl))
                return uts, vts

            def stage2(n, ci, uts, vts, ot):
                for b in range(n_yblk):
                    ps2 = ps2pool.tile([128, 512], f32, tag="ps2")
                    for si in range(4):
                        off, wd = S2_WIN[si]
                        win = ps2[:, off : off + wd]
                        ysl = slice(b * 128, (b + 1) * 128)
                        uh, ul = uts[si]
                        vh, vl = vts[si]
                        if si == 0:
                            nc.tensor.matmul(
                                ps2[:], uh[:, ysl], bt["band2h_0u_full"][:],
                                start=True, stop=False, skip_group_check=True,
                            )
                        else:
                            nc.tensor.matmul(
                                win, uh[:, ysl], bt[f"band2h_{si}"][:, 0],
                                start=False, stop=False, skip_group_check=True,
                            )
                        nc.tensor.matmul(
                            win, uh[:, ysl], bt[f"band2l_{si}"][:, 0],
                            start=False, stop=False, skip_group_check=True,
                        )
                        nc.tensor.matmul(
                            win, ul[:, ysl], bt[f"band2h_{si}"][:, 0],
                            start=False, stop=False, skip_group_check=True,
                        )
                        nc.tensor.matmul(
                            win, vh[:, ysl], bt[f"band2h_{si}"][:, 1],
                            start=False, stop=False, skip_group_check=True,
                        )
                        nc.tensor.matmul(
                            win, vh[:, ysl], bt[f"band2l_{si}"][:, 1],
                            start=False, stop=False, skip_group_check=True,
                        )
                        nc.tensor.matmul(
                            win, vl[:, ysl], bt[f"band2h_{si}"][:, 1],
                            start=False, stop=(si == 3), skip_group_check=True,
                        )
                    tmp = clpool.tile([128, 512], f32, tag="cl")
                    nc.vector.tensor_scalar(
                        tmp[:], ps2[:], 254.0, -1.0,
                        mybir.AluOpType.min, mybir.AluOpType.max,
                    )
                    nc.scalar.activation(ot[:, b, :, ci], tmp[:], copyf, bias=1.0)

            def store_out(n, ot):
                for b in range(n_yblk):
                    nc.sync.dma_start(
                        out_d.ap()[n, b * 128 : (b + 1) * 128], ot[:, b]
                    )

            # software pipeline: stage 2 lags stage 1 by one (n, ci) job
            xt = {}
            ots = {}
            pending = None
            for n in range(n_imgs):
                for ci in range(c):
                    if ci == 0:
                        xt[n] = load_x(n)
                        ots[n] = opool.tile(
                            [128, n_yblk, w, c], f32, tag="o", name=f"o_{n}"
                        )
                    uts, vts = stage1(n, ci, *xt[n])
                    if pending is not None:
                        pn, pci, puts, pvts = pending
                        stage2(pn, pci, puts, pvts, ots[pn])
                        if pci == c - 1:
                            store_out(pn, ots.pop(pn))
                            del xt[pn]
                    pending = (n, ci, uts, vts)
            pn, pci, puts, pvts = pending
            stage2(pn, pci, puts, pvts, ots[pn])
            store_out(pn, ots.pop(pn))

    nc.compile()
    return nc


_CACHE = {}


def _get_nc():
    if "nc" not in _CACHE:
        _CACHE["nc"] = build_bass()
    return _CACHE["nc"]


def kernel(x: np.ndarray) -> np.ndarray:
    from concourse import bass_utils

    nc = _get_nc()
    x = np.ascontiguousarray(x, dtype=np.float32)
    xh = x.astype(np.float16)
    xl = (x - xh.astype(np.float32)).astype(np.float16)
    const_map = dict(make_bands())
    in_maps = [
        {
            "xh": xh[k * IMG_PER_CORE : (k + 1) * IMG_PER_CORE],
            "xl": xl[k * IMG_PER_CORE : (k + 1) * IMG_PER_CORE],
            **const_map,
        }
        for k in range(N_CORES)
    ]
    res = bass_utils.run_bass_kernel_spmd(nc, in_maps, core_ids=list(range(N_CORES)))
    _CACHE["last_result"] = res
    out = np.concatenate([r["out"] for r in res.results], axis=0)
    return out.astype(np.float32)


# revision 26
# speedup vs baseline: 2.4577x; 1.1045x over previous
"""LoG as fp16 hi/lo-pair matmuls (1 cycle/row vs fp32's 4).

Two-stage banded-matrix structure; every operand is an fp16 pair
(hi + lo ~ 21 mantissa bits, enough for the +-300k pre-clip
intermediates at the 2e-2 gate):

  x  = x_hi + x_lo           (split on host; fp16 DMA halves input bytes
                              per tensor, so total input bytes match fp32)
  G  = G_hi + G_lo           (band constants, split on host)
  u  = u_hi + u_lo           (split on-chip during the stage-1 drain:
                              u_hi = fp16(psum) via ScalarE round-on-write,
                              u_lo = psum - u_hi via VectorE)

Products kept: hi*hi + hi*lo + lo*hi (lo*lo ~ 2^-22 relative, dropped).
All three matmuls of a term target the SAME psum columns and accumulate
(start only on the first matmul of the bank's group).

Stage-2's first matmul (strip 0, u_hi x G_B-hi) streams the full 512-col
row so the accumulation group's opening matmul covers the whole psum bank
(Tile/CoreSim track pending-zero state at bank granularity).

The channel loop is software-pipelined one step: stage 2 of job k runs
after stage 1 of job k+1, so the PE fills the ~1.8 us drain-latency
bubble at each stage-1 -> stage-2 transition with independent stage-1
matmuls instead of head-of-line blocking on the last strip's drains.

Input DMAs use full-128-partition row blocks: the HWDGE descriptor
splitter serializes most of a narrower transfer onto one SDMA engine;
128-partition transfers spread across all 16 engines.
"""

import numpy as np

N_CORES = 8
BATCH = 32
IMG_PER_CORE = BATCH // N_CORES
H = W = 512
C = 3
RAD = 5
TS = 102  # chunks 0-3: 102 out rows; chunk 4 absorbs the remainder (104)


def make_chunks():
    out = []
    s = 0
    while s < H:
        n = H - s if H - s <= TS + 2 else TS
        lo = min(max(0, s - RAD), H - 128)
        out.append((s, n, lo))
        s += n
    return out


CHUNKS = make_chunks()
NCH = len(CHUNKS)
# ps1 layout = exactly 2 psum banks: chunks 0-3 hold [u 102 | v 102] at
# these slots; chunk 4's u half (104) fills bank 0's gap, its v half fills
# bank 1's gap.  (2-bank ps1 frees PSUM for ps2 bufs=4, which decouples
# stage-2 bank recycling from the drain backlog in the DVE queue.)
S1_SLOT = [0, 204, 512, 716]
S1_TAIL_U = 408  # chunk 4 u -> [408, 512)
S1_TAIL_V = 920  # chunk 4 v -> [920, 1024)
S2_WIN = [(0, 133), (123, 138), (251, 138), (379, 133)]


def make_taps():
    g = np.exp(-((np.arange(3) - 1.0) ** 2) / 2.0)
    g = g / g.sum()
    S = np.array([1, 8, 28, 56, 70, 56, 28, 8, 1], dtype=np.float64)
    D2 = np.array([1, 4, 4, -4, -10, -4, 4, 4, 1], dtype=np.float64)
    return np.convolve(g, S), np.convolve(g, D2)


def scatter_matrix(taps, n=H):
    G = np.zeros((n, n), np.float64)
    for y in range(n):
        for t in range(-RAD, RAD + 1):
            src = y + t
            if src < 0:
                src = -src
            elif src > n - 1:
                src = 2 * (n - 1) - src
            G[src, y] += taps[t + RAD]
    return G


def _pair(a):
    hi = a.astype(np.float16)
    lo = (a - hi.astype(np.float64)).astype(np.float16)
    return hi, lo


def make_bands():
    A, B = make_taps()
    GA = scatter_matrix(A)
    GB = scatter_matrix(B)
    out = {}
    # stage 1: per chunk: [A win | B win] hi and lo as separate tensors
    for i, (s, n, lo_) in enumerate(CHUNKS):
        w = np.empty((128, 2 * n), np.float64)
        w[:, 0:n] = GA[lo_ : lo_ + 128, s : s + n]
        w[:, n : 2 * n] = GB[lo_ : lo_ + 128, s : s + n]
        hi, lo = _pair(w)
        out[f"band1h_{i}"] = hi
        out[f"band1l_{i}"] = lo
    # stage 2: per strip: [128, 2, w] (plane 0 = GB for u, 1 = GA for v)
    for si in range(4):
        x0 = si * 128
        off, wd = S2_WIN[si]
        wb = np.empty((128, 2, wd), np.float64)
        wb[:, 0, :] = GB[x0 : x0 + 128, off : off + wd]
        wb[:, 1, :] = GA[x0 : x0 + 128, off : off + wd]
        for G in (GA, GB):
            assert abs(G[x0 : x0 + 128, :off]).max(initial=0) == 0
            assert abs(G[x0 : x0 + 128, off + wd :]).max(initial=0) == 0
        hi, lo = _pair(wb)
        out[f"band2h_{si}"] = hi
        out[f"band2l_{si}"] = lo
    # strip 0's u-hi matmul streams the full 512-col row (group opener)
    w = np.zeros((128, 512), np.float64)
    w[:, 0:133] = GB[0:128, 0:133]
    out["band2h_0u_full"] = _pair(w)[0]
    return out


def build_bass(n_imgs=IMG_PER_CORE, h=H, w=W, c=C):
    import concourse.bacc as bacc
    import concourse.mybir as mybir
    import concourse.tile as tile

    f32 = mybir.dt.float32
    f16 = mybir.dt.float16
    copyf = mybir.ActivationFunctionType.Copy

    nc = bacc.Bacc("TRN2", target_bir_lowering=False, debug=False)
    xh_d = nc.dram_tensor("xh", [n_imgs, h, w, c], f16, kind="ExternalInput")
    xl_d = nc.dram_tensor("xl", [n_imgs, h, w, c], f16, kind="ExternalInput")
    out_d = nc.dram_tensor("out", [n_imgs, h, w, c], f32, kind="ExternalOutput")
    bands = make_bands()
    band_d = {
        k: nc.dram_tensor(k, list(v.shape), f16, kind="ExternalInput")
        for k, v in bands.items()
    }

    n_yblk = h // 128
    tail = CHUNKS[4][1]  # 40

    with tile.TileContext(nc) as tc:
        with (
            tc.tile_pool(name="const", bufs=1) as cpool,
            tc.tile_pool(name="xin", bufs=2) as xpool,
            tc.tile_pool(name="uv", bufs=2) as uvpool,
            tc.tile_pool(name="outp", bufs=2) as opool,
            tc.tile_pool(name="clip", bufs=3) as clpool,
            tc.tile_pool(name="ps1", bufs=2, space="PSUM") as ps1pool,
            tc.tile_pool(name="ps2", bufs=4, space="PSUM") as ps2pool,
        ):
            bt = {}
            for k, v in bands.items():
                t = cpool.tile(list(v.shape), f16, name=k)
                nc.sync.dma_start(t[:], band_d[k].ap())
                bt[k] = t

            def load_x(n):
                xhr, xlr = [], []
                for i, (s, nn_, lo) in enumerate(CHUNKS):
                    th = xpool.tile([128, w, c], f16, tag=f"xh{i}", name=f"xh{i}_{n}")
                    nc.sync.dma_start(th[:], xh_d.ap()[n, lo : lo + 128])
                    xhr.append(th)
                    tl = xpool.tile([128, w, c], f16, tag=f"xl{i}", name=f"xl{i}_{n}")
                    nc.sync.dma_start(tl[:], xl_d.ap()[n, lo : lo + 128])
                    xlr.append(tl)
                return xhr, xlr

            def stage1(n, ci, xhr, xlr):
                uts, vts = [], []
                nt = CHUNKS[4][1]
                for si in range(4):
                    ps1 = ps1pool.tile([128, 1024], f32, tag="ps1")
                    for i in range(NCH):
                        xh = xhr[i][:, si * 128 : (si + 1) * 128, ci]
                        xl = xlr[i][:, si * 128 : (si + 1) * 128, ci]
                        if i < 4:
                            # one fused [u | v] 204-col slot, 3 matmuls
                            dst = ps1[:, S1_SLOT[i] : S1_SLOT[i] + 2 * TS]
                            parts = ((dst, slice(0, 2 * TS)),)
                        else:
                            # chunk 4: u and v halves live in different banks
                            parts = (
                                (ps1[:, S1_TAIL_U : S1_TAIL_U + nt], slice(0, nt)),
                                (ps1[:, S1_TAIL_V : S1_TAIL_V + nt], slice(nt, 2 * nt)),
                            )
                        for dst, bsl in parts:
                            nc.tensor.matmul(
                                dst, xh, bt[f"band1h_{i}"][:, bsl],
                                start=True, stop=False, skip_group_check=True,
                            )
                            nc.tensor.matmul(
                                dst, xh, bt[f"band1l_{i}"][:, bsl],
                                start=False, stop=False, skip_group_check=True,
                            )
                            nc.tensor.matmul(
                                dst, xl, bt[f"band1h_{i}"][:, bsl],
                                start=False, stop=True, skip_group_check=True,
                            )
                    # drains: u_hi = fp16(psum) on ScalarE (round on write),
                    # u_lo = psum - u_hi on VectorE.
                    uh = uvpool.tile([128, h], f16, tag=f"uh{si}", name=f"uh{si}_{n}_{ci}")
                    ul = uvpool.tile([128, h], f16, tag=f"ul{si}", name=f"ul{si}_{n}_{ci}")
                    vh = uvpool.tile([128, h], f16, tag=f"vh{si}", name=f"vh{si}_{n}_{ci}")
                    vl = uvpool.tile([128, h], f16, tag=f"vl{si}", name=f"vl{si}_{n}_{ci}")
                    # [p, bank 2, chunk-pair 2, 204] view of the slot region
                    slots = (
                        ps1[:]
                        .rearrange("p (bk y) -> p bk y", bk=2)[:, :, 0 : 2 * 2 * TS]
                        .rearrange("p bk (ch x) -> p bk ch x", ch=2)
                    )
                    for hi_t, lo_t, poff, toff in (
                        (uh, ul, 0, S1_TAIL_U),
                        (vh, vl, TS, S1_TAIL_V),
                    ):
                        src = slots[:, :, :, poff : poff + TS]
                        hi4 = hi_t[:, 0 : 4 * TS].rearrange(
                            "p (bk ch x) -> p bk ch x", bk=2, ch=2
                        )
                        nc.scalar.copy(hi4, src)
                        nc.vector.tensor_tensor(
                            lo_t[:, 0 : 4 * TS].rearrange(
                                "p (bk ch x) -> p bk ch x", bk=2, ch=2
                            ),
                            src, hi4, mybir.AluOpType.subtract,
                        )
                        tsrc = ps1[:, toff : toff + tail]
                        nc.scalar.copy(hi_t[:, 4 * TS : 4 * TS + tail], tsrc)
                        nc.vector.tensor_tensor(
                            lo_t[:, 4 * TS : 4 * TS + tail],
                            tsrc, hi_t[:, 4 * TS : 4 * TS + tail],
                            mybir.AluOpType.subtract,
                        )
                    uts.append((uh, ul))
                    vts.append((vh, vl))
                return uts, vts

            def stage2(n, ci, uts, vts, ot):
                for b in range(n_yblk):
                    ps2 = ps2pool.tile([128, 512], f32, tag="ps2")
                    for si in range(4):
                        off, wd = S2_WIN[si]
                        win = ps2[:, off : off + wd]
                        ysl = slice(b * 128, (b + 1) * 128)
                        uh, ul = uts[si]
                        vh, vl = vts[si]
                        if si == 0:
                            nc.tensor.matmul(
                                ps2[:], uh[:, ysl], bt["band2h_0u_full"][:],
                                start=True, stop=False, skip_group_check=True,
                            )
                        else:
                            nc.tensor.matmul(
                                win, uh[:, ysl], bt[f"band2h_{si}"][:, 0],
                                start=False, stop=False, skip_group_check=True,
                            )
                        nc.tensor.matmul(
                            win, uh[:, ysl], bt[f"band2l_{si}"][:, 0],
                            start=False, stop=False, skip_group_check=True,
                        )
                        nc.tensor.matmul(
                            win, ul[:, ysl], bt[f"band2h_{si}"][:, 0],
                            start=False, stop=False, skip_group_check=True,
                        )
                        nc.tensor.matmul(
                            win, vh[:, ysl], bt[f"band2h_{si}"][:, 1],
                            start=False, stop=False, skip_group_check=True,
                        )
                        nc.tensor.matmul(
                            win, vh[:, ysl], bt[f"band2l_{si}"][:, 1],
                            start=False, stop=False, skip_group_check=True,
                        )
                        nc.tensor.matmul(
                            win, vl[:, ysl], bt[f"band2h_{si}"][:, 1],
                            start=False, stop=(si == 3), skip_group_check=True,
                        )
                    tmp = clpool.tile([128, 512], f32, tag="cl")
                    nc.vector.tensor_scalar(
                        tmp[:], ps2[:], 254.0, -1.0,
                        mybir.AluOpType.min, mybir.AluOpType.max,
                    )
                    nc.scalar.activation(ot[:, b, :, ci], tmp[:], copyf, bias=1.0)

            def store_out(n, ot):
                for b in range(n_yblk):
                    nc.sync.dma_start(
                        out_d.ap()[n, b * 128 : (b + 1) * 128], ot[:, b]
                    )

            # software pipeline: stage 2 lags stage 1 by one (n, ci) job
            xt = {}
            ots = {}
            pending = None
            for n in range(n_imgs):
                for ci in range(c):
                    if ci == 0:
                        xt[n] = load_x(n)
                        ots[n] = opool.tile(
                            [128, n_yblk, w, c], f32, tag="o", name=f"o_{n}"
                        )
                    uts, vts = stage1(n, ci, *xt[n])
                    if pending is not None:
                        pn, pci, puts, pvts = pending
                        stage2(pn, pci, puts, pvts, ots[pn])
                        if pci == c - 1:
                            store_out(pn, ots.pop(pn))
                            del xt[pn]
                    pending = (n, ci, uts, vts)
            pn, pci, puts, pvts = pending
            stage2(pn, pci, puts, pvts, ots[pn])
            store_out(pn, ots.pop(pn))

    nc.compile()
    return nc


_CACHE = {}


def _get_nc():
    if "nc" not in _CACHE:
        _CACHE["nc"] = build_bass()
    return _CACHE["nc"]


def kernel(x: np.ndarray) -> np.ndarray:
    from concourse import bass_utils

    nc = _get_nc()
    x = np.ascontiguousarray(x, dtype=np.float32)
    xh = x.astype(np.float16)
    xl = (x - xh.astype(np.float32)).astype(np.float16)
    const_map = dict(make_bands())
    in_maps = [
        {
            "xh": xh[k * IMG_PER_CORE : (k + 1) * IMG_PER_CORE],
            "xl": xl[k * IMG_PER_CORE : (k + 1) * IMG_PER_CORE],
            **const_map,
        }
        for k in range(N_CORES)
    ]
    res = bass_utils.run_bass_kernel_spmd(nc, in_maps, core_ids=list(range(N_CORES)))
    _CACHE["last_result"] = res
    out = np.concatenate([r["out"] for r in res.results], axis=0)
    return out.astype(np.float32)


# revision 28
# speedup vs baseline: 2.5085x; 1.0207x over previous
"""LoG as fp16 hi/lo-pair matmuls (1 cycle/row vs fp32's 4).

Two-stage banded-matrix structure; every operand is an fp16 pair
(hi + lo ~ 21 mantissa bits, enough for the +-300k pre-clip
intermediates at the 2e-2 gate):

  x  = x_hi + x_lo           (split on host; fp16 DMA halves input bytes
                              per tensor, so total input bytes match fp32)
  G  = G_hi + G_lo           (band constants, split on host)
  u  = u_hi + u_lo           (split on-chip during the stage-1 drain:
                              u_hi = fp16(psum) via ScalarE round-on-write,
                              u_lo = psum - u_hi via VectorE)

Products kept: hi*hi + hi*lo + lo*hi (lo*lo ~ 2^-22 relative, dropped).
All three matmuls of a term target the SAME psum columns and accumulate
(start only on the first matmul of the bank's group).

Stage-2's first matmul (strip 0, u_hi x G_B-hi) streams the full 512-col
row so the accumulation group's opening matmul covers the whole psum bank
(Tile/CoreSim track pending-zero state at bank granularity).

The channel loop is software-pipelined one step: stage 2 of job k runs
after stage 1 of job k+1, so the PE fills the ~1.8 us drain-latency
bubble at each stage-1 -> stage-2 transition with independent stage-1
matmuls instead of head-of-line blocking on the last strip's drains.

Input DMAs use full-128-partition row blocks: the HWDGE descriptor
splitter serializes most of a narrower transfer onto one SDMA engine;
128-partition transfers spread across all 16 engines.
"""

import numpy as np

N_CORES = 8
BATCH = 32
IMG_PER_CORE = BATCH // N_CORES
H = W = 512
C = 3
RAD = 5
TS = 102  # chunks 0-3: 102 out rows; chunk 4 absorbs the remainder (104)


def make_chunks():
    out = []
    s = 0
    while s < H:
        n = H - s if H - s <= TS + 2 else TS
        lo = min(max(0, s - RAD), H - 128)
        out.append((s, n, lo))
        s += n
    return out


CHUNKS = make_chunks()
NCH = len(CHUNKS)
# ps1 layout = exactly 2 psum banks: chunks 0-3 hold [u 102 | v 102] at
# these slots; chunk 4's u half (104) fills bank 0's gap, its v half fills
# bank 1's gap.  (2-bank ps1 frees PSUM for ps2 bufs=4, which decouples
# stage-2 bank recycling from the drain backlog in the DVE queue.)
S1_SLOT = [0, 204, 512, 716]
S1_TAIL_U = 408  # chunk 4 u -> [408, 512)
S1_TAIL_V = 920  # chunk 4 v -> [920, 1024)
S2_WIN = [(0, 133), (123, 138), (251, 138), (379, 133)]


def make_taps():
    g = np.exp(-((np.arange(3) - 1.0) ** 2) / 2.0)
    g = g / g.sum()
    S = np.array([1, 8, 28, 56, 70, 56, 28, 8, 1], dtype=np.float64)
    D2 = np.array([1, 4, 4, -4, -10, -4, 4, 4, 1], dtype=np.float64)
    return np.convolve(g, S), np.convolve(g, D2)


def scatter_matrix(taps, n=H):
    G = np.zeros((n, n), np.float64)
    for y in range(n):
        for t in range(-RAD, RAD + 1):
            src = y + t
            if src < 0:
                src = -src
            elif src > n - 1:
                src = 2 * (n - 1) - src
            G[src, y] += taps[t + RAD]
    return G


def _pair(a):
    hi = a.astype(np.float16)
    lo = (a - hi.astype(np.float64)).astype(np.float16)
    return hi, lo


def make_bands():
    A, B = make_taps()
    GA = scatter_matrix(A)
    GB = scatter_matrix(B)
    out = {}
    # stage 1: per chunk: [A win | B win] hi and lo as separate tensors
    for i, (s, n, lo_) in enumerate(CHUNKS):
        w = np.empty((128, 2 * n), np.float64)
        w[:, 0:n] = GA[lo_ : lo_ + 128, s : s + n]
        w[:, n : 2 * n] = GB[lo_ : lo_ + 128, s : s + n]
        hi, lo = _pair(w)
        out[f"band1h_{i}"] = hi
        out[f"band1l_{i}"] = lo
    # stage 2: per strip: [128, 2, w] (plane 0 = GB for u, 1 = GA for v)
    for si in range(4):
        x0 = si * 128
        off, wd = S2_WIN[si]
        wb = np.empty((128, 2, wd), np.float64)
        wb[:, 0, :] = GB[x0 : x0 + 128, off : off + wd]
        wb[:, 1, :] = GA[x0 : x0 + 128, off : off + wd]
        for G in (GA, GB):
            assert abs(G[x0 : x0 + 128, :off]).max(initial=0) == 0
            assert abs(G[x0 : x0 + 128, off + wd :]).max(initial=0) == 0
        hi, lo = _pair(wb)
        out[f"band2h_{si}"] = hi
        out[f"band2l_{si}"] = lo
    # strip 0's u-hi matmul streams the full 512-col row (group opener)
    w = np.zeros((128, 512), np.float64)
    w[:, 0:133] = GB[0:128, 0:133]
    out["band2h_0u_full"] = _pair(w)[0]
    return out


def build_bass(n_imgs=IMG_PER_CORE, h=H, w=W, c=C):
    import concourse.bacc as bacc
    import concourse.mybir as mybir
    import concourse.tile as tile

    f32 = mybir.dt.float32
    f16 = mybir.dt.float16
    copyf = mybir.ActivationFunctionType.Copy

    nc = bacc.Bacc("TRN2", target_bir_lowering=False, debug=False)
    xh_d = nc.dram_tensor("xh", [n_imgs, h, w, c], f16, kind="ExternalInput")
    xl_d = nc.dram_tensor("xl", [n_imgs, h, w, c], f16, kind="ExternalInput")
    out_d = nc.dram_tensor("out", [n_imgs, h, w, c], f32, kind="ExternalOutput")
    bands = make_bands()
    band_d = {
        k: nc.dram_tensor(k, list(v.shape), f16, kind="ExternalInput")
        for k, v in bands.items()
    }

    n_yblk = h // 128
    tail = CHUNKS[4][1]  # 40

    with tile.TileContext(nc) as tc:
        with (
            tc.tile_pool(name="const", bufs=1) as cpool,
            tc.tile_pool(name="xin", bufs=2) as xpool,
            tc.tile_pool(name="uv", bufs=3) as uvpool,
            tc.tile_pool(name="outp", bufs=2) as opool,
            tc.tile_pool(name="clip", bufs=3) as clpool,
            tc.tile_pool(name="ps1", bufs=2, space="PSUM") as ps1pool,
            tc.tile_pool(name="ps2", bufs=4, space="PSUM") as ps2pool,
        ):
            bt = {}
            for k, v in bands.items():
                t = cpool.tile(list(v.shape), f16, name=k)
                nc.sync.dma_start(t[:], band_d[k].ap())
                bt[k] = t

            def load_x(n):
                xhr, xlr = [], []
                for i, (s, nn_, lo) in enumerate(CHUNKS):
                    th = xpool.tile([128, w, c], f16, tag=f"xh{i}", name=f"xh{i}_{n}")
                    nc.sync.dma_start(th[:], xh_d.ap()[n, lo : lo + 128])
                    xhr.append(th)
                    tl = xpool.tile([128, w, c], f16, tag=f"xl{i}", name=f"xl{i}_{n}")
                    nc.sync.dma_start(tl[:], xl_d.ap()[n, lo : lo + 128])
                    xlr.append(tl)
                return xhr, xlr

            def stage1(n, ci, xhr, xlr):
                uts, vts = [], []
                nt = CHUNKS[4][1]
                for si in range(4):
                    ps1 = ps1pool.tile([128, 1024], f32, tag="ps1")
                    for i in range(NCH):
                        xh = xhr[i][:, si * 128 : (si + 1) * 128, ci]
                        xl = xlr[i][:, si * 128 : (si + 1) * 128, ci]
                        if i < 4:
                            # one fused [u | v] 204-col slot, 3 matmuls
                            dst = ps1[:, S1_SLOT[i] : S1_SLOT[i] + 2 * TS]
                            parts = ((dst, slice(0, 2 * TS)),)
                        else:
                            # chunk 4: u and v halves live in different banks
                            parts = (
                                (ps1[:, S1_TAIL_U : S1_TAIL_U + nt], slice(0, nt)),
                                (ps1[:, S1_TAIL_V : S1_TAIL_V + nt], slice(nt, 2 * nt)),
                            )
                        for dst, bsl in parts:
                            nc.tensor.matmul(
                                dst, xh, bt[f"band1h_{i}"][:, bsl],
                                start=True, stop=False, skip_group_check=True,
                            )
                            nc.tensor.matmul(
                                dst, xh, bt[f"band1l_{i}"][:, bsl],
                                start=False, stop=False, skip_group_check=True,
                            )
                            nc.tensor.matmul(
                                dst, xl, bt[f"band1h_{i}"][:, bsl],
                                start=False, stop=True, skip_group_check=True,
                            )
                    # drains: u_hi = fp16(psum) on ScalarE (round on write),
                    # u_lo = psum - u_hi on VectorE.
                    uh = uvpool.tile([128, h], f16, tag=f"uh{si}", name=f"uh{si}_{n}_{ci}")
                    ul = uvpool.tile([128, h], f16, tag=f"ul{si}", name=f"ul{si}_{n}_{ci}")
                    vh = uvpool.tile([128, h], f16, tag=f"vh{si}", name=f"vh{si}_{n}_{ci}")
                    vl = uvpool.tile([128, h], f16, tag=f"vl{si}", name=f"vl{si}_{n}_{ci}")
                    # [p, bank 2, chunk-pair 2, 204] view of the slot region
                    slots = (
                        ps1[:]
                        .rearrange("p (bk y) -> p bk y", bk=2)[:, :, 0 : 2 * 2 * TS]
                        .rearrange("p bk (ch x) -> p bk ch x", ch=2)
                    )
                    for hi_t, lo_t, poff, toff in (
                        (uh, ul, 0, S1_TAIL_U),
                        (vh, vl, TS, S1_TAIL_V),
                    ):
                        src = slots[:, :, :, poff : poff + TS]
                        hi4 = hi_t[:, 0 : 4 * TS].rearrange(
                            "p (bk ch x) -> p bk ch x", bk=2, ch=2
                        )
                        nc.scalar.copy(hi4, src)
                        nc.vector.tensor_tensor(
                            lo_t[:, 0 : 4 * TS].rearrange(
                                "p (bk ch x) -> p bk ch x", bk=2, ch=2
                            ),
                            src, hi4, mybir.AluOpType.subtract,
                        )
                        tsrc = ps1[:, toff : toff + tail]
                        nc.scalar.copy(hi_t[:, 4 * TS : 4 * TS + tail], tsrc)
                        nc.vector.tensor_tensor(
                            lo_t[:, 4 * TS : 4 * TS + tail],
                            tsrc, hi_t[:, 4 * TS : 4 * TS + tail],
                            mybir.AluOpType.subtract,
                        )
                    uts.append((uh, ul))
                    vts.append((vh, vl))
                return uts, vts

            def stage2(n, ci, uts, vts, ot):
                for b in range(n_yblk):
                    ps2 = ps2pool.tile([128, 512], f32, tag="ps2")
                    for si in range(4):
                        off, wd = S2_WIN[si]
                        win = ps2[:, off : off + wd]
                        ysl = slice(b * 128, (b + 1) * 128)
                        uh, ul = uts[si]
                        vh, vl = vts[si]
                        if si == 0:
                            nc.tensor.matmul(
                                ps2[:], uh[:, ysl], bt["band2h_0u_full"][:],
                                start=True, stop=False, skip_group_check=True,
                            )
                        else:
                            nc.tensor.matmul(
                                win, uh[:, ysl], bt[f"band2h_{si}"][:, 0],
                                start=False, stop=False, skip_group_check=True,
                            )
                        nc.tensor.matmul(
                            win, uh[:, ysl], bt[f"band2l_{si}"][:, 0],
                            start=False, stop=False, skip_group_check=True,
                        )
                        nc.tensor.matmul(
                            win, ul[:, ysl], bt[f"band2h_{si}"][:, 0],
                            start=False, stop=False, skip_group_check=True,
                        )
                        nc.tensor.matmul(
                            win, vh[:, ysl], bt[f"band2h_{si}"][:, 1],
                            start=False, stop=False, skip_group_check=True,
                        )
                        nc.tensor.matmul(
                            win, vh[:, ysl], bt[f"band2l_{si}"][:, 1],
                            start=False, stop=False, skip_group_check=True,
                        )
                        nc.tensor.matmul(
                            win, vl[:, ysl], bt[f"band2h_{si}"][:, 1],
                            start=False, stop=(si == 3), skip_group_check=True,
                        )
                    tmp = clpool.tile([128, 512], f32, tag="cl")
                    nc.vector.tensor_scalar(
                        tmp[:], ps2[:], 254.0, -1.0,
                        mybir.AluOpType.min, mybir.AluOpType.max,
                    )
                    nc.scalar.activation(ot[:, b, :, ci], tmp[:], copyf, bias=1.0)

            def store_out(n, ot):
                for b in range(n_yblk):
                    nc.sync.dma_start(
                        out_d.ap()[n, b * 128 : (b + 1) * 128], ot[:, b]
                    )

            # software pipeline: stage 2 lags stage 1 by two (n, ci) jobs
            # (uv pool bufs=3 holds the three in-flight generations), so the
            # PE never waits on the drain/clamp tail of an adjacent job.
            LAG = 2

            def run_stage2(job):
                pn, pci, puts, pvts = job
                stage2(pn, pci, puts, pvts, ots[pn])
                if pci == c - 1:
                    store_out(pn, ots.pop(pn))
                    del xt[pn]

            xt = {}
            ots = {}
            pending = []
            for n in range(n_imgs):
                for ci in range(c):
                    if ci == 0:
                        xt[n] = load_x(n)
                        ots[n] = opool.tile(
                            [128, n_yblk, w, c], f32, tag="o", name=f"o_{n}"
                        )
                    uts, vts = stage1(n, ci, *xt[n])
                    pending.append((n, ci, uts, vts))
                    if len(pending) > LAG:
                        run_stage2(pending.pop(0))
            for job in pending:
                run_stage2(job)

    nc.compile()
    return nc


_CACHE = {}


def _get_nc():
    if "nc" not in _CACHE:
        _CACHE["nc"] = build_bass()
    return _CACHE["nc"]


def kernel(x: np.ndarray) -> np.ndarray:
    from concourse import bass_utils

    nc = _get_nc()
    x = np.ascontiguousarray(x, dtype=np.float32)
    xh = x.astype(np.float16)
    xl = (x - xh.astype(np.float32)).astype(np.float16)
    const_map = dict(make_bands())
    in_maps = [
        {
            "xh": xh[k * IMG_PER_CORE : (k + 1) * IMG_PER_CORE],
            "xl": xl[k * IMG_PER_CORE : (k + 1) * IMG_PER_CORE],
            **const_map,
        }
        for k in range(N_CORES)
    ]
    res = bass_utils.run_bass_kernel_spmd(nc, in_maps, core_ids=list(range(N_CORES)))
    _CACHE["last_result"] = res
    out = np.concatenate([r["out"] for r in res.results], axis=0)
    return out.astype(np.float32)


# revision 30
# speedup vs baseline: 2.5696x; 1.0243x over previous
"""LoG as fp16 hi/lo-pair matmuls (1 cycle/row vs fp32's 4).

Two-stage banded-matrix structure; every operand is an fp16 pair
(hi + lo ~ 21 mantissa bits, enough for the +-300k pre-clip
intermediates at the 2e-2 gate):

  x  = x_hi + x_lo           (split on host; fp16 DMA halves input bytes
                              per tensor, so total input bytes match fp32)
  G  = G_hi + G_lo           (band constants, split on host)
  u  = u_hi + u_lo           (split on-chip during the stage-1 drain:
                              u_hi = fp16(psum) via ScalarE round-on-write,
                              u_lo = psum - u_hi via VectorE)

Products kept: hi*hi + hi*lo + lo*hi (lo*lo ~ 2^-22 relative, dropped).
All three matmuls of a term target the SAME psum columns and accumulate
(start only on the first matmul of the bank's group).

Stage-2's first matmul (strip 0, u_hi x G_B-hi) streams the full 512-col
row so the accumulation group's opening matmul covers the whole psum bank
(Tile/CoreSim track pending-zero state at bank granularity).

The channel loop is software-pipelined one step: stage 2 of job k runs
after stage 1 of job k+1, so the PE fills the ~1.8 us drain-latency
bubble at each stage-1 -> stage-2 transition with independent stage-1
matmuls instead of head-of-line blocking on the last strip's drains.

Input DMAs use full-128-partition row blocks: the HWDGE descriptor
splitter serializes most of a narrower transfer onto one SDMA engine;
128-partition transfers spread across all 16 engines.
"""

import numpy as np

N_CORES = 8
BATCH = 32
IMG_PER_CORE = BATCH // N_CORES
H = W = 512
C = 3
RAD = 5
TS = 102  # chunks 0-3: 102 out rows; chunk 4 absorbs the remainder (104)


def make_chunks():
    out = []
    s = 0
    while s < H:
        n = H - s if H - s <= TS + 2 else TS
        lo = min(max(0, s - RAD), H - 128)
        out.append((s, n, lo))
        s += n
    return out


CHUNKS = make_chunks()
NCH = len(CHUNKS)
# ps1 layout = exactly 2 psum banks: chunks 0-3 hold [u 102 | v 102] at
# these slots; chunk 4's u half (104) fills bank 0's gap, its v half fills
# bank 1's gap.  (2-bank ps1 frees PSUM for ps2 bufs=4, which decouples
# stage-2 bank recycling from the drain backlog in the DVE queue.)
S1_SLOT = [0, 204, 512, 716]
S1_TAIL_U = 408  # chunk 4 u -> [408, 512)
S1_TAIL_V = 920  # chunk 4 v -> [920, 1024)
S2_WIN = [(0, 133), (123, 138), (251, 138), (379, 133)]


def make_taps():
    g = np.exp(-((np.arange(3) - 1.0) ** 2) / 2.0)
    g = g / g.sum()
    S = np.array([1, 8, 28, 56, 70, 56, 28, 8, 1], dtype=np.float64)
    D2 = np.array([1, 4, 4, -4, -10, -4, 4, 4, 1], dtype=np.float64)
    return np.convolve(g, S), np.convolve(g, D2)


def scatter_matrix(taps, n=H):
    G = np.zeros((n, n), np.float64)
    for y in range(n):
        for t in range(-RAD, RAD + 1):
            src = y + t
            if src < 0:
                src = -src
            elif src > n - 1:
                src = 2 * (n - 1) - src
            G[src, y] += taps[t + RAD]
    return G


def _pair(a):
    hi = a.astype(np.float16)
    lo = (a - hi.astype(np.float64)).astype(np.float16)
    return hi, lo


def make_bands():
    A, B = make_taps()
    GA = scatter_matrix(A)
    GB = scatter_matrix(B)
    out = {}
    # stage 1: per chunk: [A win | B win] hi and lo as separate tensors
    for i, (s, n, lo_) in enumerate(CHUNKS):
        w = np.empty((128, 2 * n), np.float64)
        w[:, 0:n] = GA[lo_ : lo_ + 128, s : s + n]
        w[:, n : 2 * n] = GB[lo_ : lo_ + 128, s : s + n]
        hi, lo = _pair(w)
        out[f"band1h_{i}"] = hi
        out[f"band1l_{i}"] = lo
    # stage 2: per strip: [128, 2, w] (plane 0 = GB for u, 1 = GA for v)
    for si in range(4):
        x0 = si * 128
        off, wd = S2_WIN[si]
        wb = np.empty((128, 2, wd), np.float64)
        wb[:, 0, :] = GB[x0 : x0 + 128, off : off + wd]
        wb[:, 1, :] = GA[x0 : x0 + 128, off : off + wd]
        for G in (GA, GB):
            assert abs(G[x0 : x0 + 128, :off]).max(initial=0) == 0
            assert abs(G[x0 : x0 + 128, off + wd :]).max(initial=0) == 0
        hi, lo = _pair(wb)
        out[f"band2h_{si}"] = hi
        out[f"band2l_{si}"] = lo
    # strip 0's u-hi matmul streams the full 512-col row (group opener)
    w = np.zeros((128, 512), np.float64)
    w[:, 0:133] = GB[0:128, 0:133]
    out["band2h_0u_full"] = _pair(w)[0]
    return out


def build_bass(n_imgs=IMG_PER_CORE, h=H, w=W, c=C):
    import concourse.bacc as bacc
    import concourse.mybir as mybir
    import concourse.tile as tile

    f32 = mybir.dt.float32
    f16 = mybir.dt.float16
    copyf = mybir.ActivationFunctionType.Copy

    nc = bacc.Bacc("TRN2", target_bir_lowering=False, debug=False)
    xh_d = nc.dram_tensor("xh", [n_imgs, h, w, c], f16, kind="ExternalInput")
    xl_d = nc.dram_tensor("xl", [n_imgs, h, w, c], f16, kind="ExternalInput")
    out_d = nc.dram_tensor("out", [n_imgs, h, w, c], f32, kind="ExternalOutput")
    bands = make_bands()
    band_d = {
        k: nc.dram_tensor(k, list(v.shape), f16, kind="ExternalInput")
        for k, v in bands.items()
    }

    n_yblk = h // 128
    tail = CHUNKS[4][1]  # 40

    with tile.TileContext(nc) as tc:
        with (
            tc.tile_pool(name="const", bufs=1) as cpool,
            tc.tile_pool(name="xin", bufs=2) as xpool,
            tc.tile_pool(name="uv", bufs=3) as uvpool,
            tc.tile_pool(name="outp", bufs=2) as opool,
            tc.tile_pool(name="clip", bufs=3) as clpool,
            tc.tile_pool(name="ps1", bufs=2, space="PSUM") as ps1pool,
            tc.tile_pool(name="ps2", bufs=4, space="PSUM") as ps2pool,
        ):
            bt = {}

            def load_band(k):
                t = cpool.tile(list(bands[k].shape), f16, name=k)
                nc.sync.dma_start(t[:], band_d[k].ap())
                bt[k] = t

            def load_x(n):
                xhr, xlr = [], []
                for i, (s, nn_, lo) in enumerate(CHUNKS):
                    th = xpool.tile([128, w, c], f16, tag=f"xh{i}", name=f"xh{i}_{n}")
                    nc.sync.dma_start(th[:], xh_d.ap()[n, lo : lo + 128])
                    xhr.append(th)
                    tl = xpool.tile([128, w, c], f16, tag=f"xl{i}", name=f"xl{i}_{n}")
                    nc.sync.dma_start(tl[:], xl_d.ap()[n, lo : lo + 128])
                    xlr.append(tl)
                    # interleave stage-1 band loads with image 0's chunks so
                    # the first matmuls aren't queued behind all 21 band
                    # transfers in the HWDGE FIFO (first-MM was at 26.4 us).
                    if n == 0:
                        load_band(f"band1h_{i}")
                        load_band(f"band1l_{i}")
                return xhr, xlr

            def load_stage2_bands():
                # needed only once stage 2 of job 0 runs (two jobs in)
                for si in range(4):
                    load_band(f"band2h_{si}")
                    load_band(f"band2l_{si}")
                load_band("band2h_0u_full")

            def stage1(n, ci, xhr, xlr):
                uts, vts = [], []
                nt = CHUNKS[4][1]
                for si in range(4):
                    ps1 = ps1pool.tile([128, 1024], f32, tag="ps1")
                    for i in range(NCH):
                        xh = xhr[i][:, si * 128 : (si + 1) * 128, ci]
                        xl = xlr[i][:, si * 128 : (si + 1) * 128, ci]
                        if i < 4:
                            # one fused [u | v] 204-col slot, 3 matmuls
                            dst = ps1[:, S1_SLOT[i] : S1_SLOT[i] + 2 * TS]
                            parts = ((dst, slice(0, 2 * TS)),)
                        else:
                            # chunk 4: u and v halves live in different banks
                            parts = (
                                (ps1[:, S1_TAIL_U : S1_TAIL_U + nt], slice(0, nt)),
                                (ps1[:, S1_TAIL_V : S1_TAIL_V + nt], slice(nt, 2 * nt)),
                            )
                        for dst, bsl in parts:
                            nc.tensor.matmul(
                                dst, xh, bt[f"band1h_{i}"][:, bsl],
                                start=True, stop=False, skip_group_check=True,
                            )
                            nc.tensor.matmul(
                                dst, xh, bt[f"band1l_{i}"][:, bsl],
                                start=False, stop=False, skip_group_check=True,
                            )
                            nc.tensor.matmul(
                                dst, xl, bt[f"band1h_{i}"][:, bsl],
                                start=False, stop=True, skip_group_check=True,
                            )
                    # drains: u_hi = fp16(psum) on ScalarE (round on write),
                    # u_lo = psum - u_hi on VectorE.
                    uh = uvpool.tile([128, h], f16, tag=f"uh{si}", name=f"uh{si}_{n}_{ci}")
                    ul = uvpool.tile([128, h], f16, tag=f"ul{si}", name=f"ul{si}_{n}_{ci}")
                    vh = uvpool.tile([128, h], f16, tag=f"vh{si}", name=f"vh{si}_{n}_{ci}")
                    vl = uvpool.tile([128, h], f16, tag=f"vl{si}", name=f"vl{si}_{n}_{ci}")
                    # [p, bank 2, chunk-pair 2, 204] view of the slot region
                    slots = (
                        ps1[:]
                        .rearrange("p (bk y) -> p bk y", bk=2)[:, :, 0 : 2 * 2 * TS]
                        .rearrange("p bk (ch x) -> p bk ch x", ch=2)
                    )
                    for hi_t, lo_t, poff, toff in (
                        (uh, ul, 0, S1_TAIL_U),
                        (vh, vl, TS, S1_TAIL_V),
                    ):
                        src = slots[:, :, :, poff : poff + TS]
                        hi4 = hi_t[:, 0 : 4 * TS].rearrange(
                            "p (bk ch x) -> p bk ch x", bk=2, ch=2
                        )
                        nc.scalar.copy(hi4, src)
                        nc.vector.tensor_tensor(
                            lo_t[:, 0 : 4 * TS].rearrange(
                                "p (bk ch x) -> p bk ch x", bk=2, ch=2
                            ),
                            src, hi4, mybir.AluOpType.subtract,
                        )
                        tsrc = ps1[:, toff : toff + tail]
                        nc.scalar.copy(hi_t[:, 4 * TS : 4 * TS + tail], tsrc)
                        nc.vector.tensor_tensor(
                            lo_t[:, 4 * TS : 4 * TS + tail],
                            tsrc, hi_t[:, 4 * TS : 4 * TS + tail],
                            mybir.AluOpType.subtract,
                        )
                    uts.append((uh, ul))
                    vts.append((vh, vl))
                return uts, vts

            def stage2(n, ci, uts, vts, ot):
                for b in range(n_yblk):
                    ps2 = ps2pool.tile([128, 512], f32, tag="ps2")
                    for si in range(4):
                        off, wd = S2_WIN[si]
                        win = ps2[:, off : off + wd]
                        ysl = slice(b * 128, (b + 1) * 128)
                        uh, ul = uts[si]
                        vh, vl = vts[si]
                        if si == 0:
                            nc.tensor.matmul(
                                ps2[:], uh[:, ysl], bt["band2h_0u_full"][:],
                                start=True, stop=False, skip_group_check=True,
                            )
                        else:
                            nc.tensor.matmul(
                                win, uh[:, ysl], bt[f"band2h_{si}"][:, 0],
                                start=False, stop=False, skip_group_check=True,
                            )
                        nc.tensor.matmul(
                            win, uh[:, ysl], bt[f"band2l_{si}"][:, 0],
                            start=False, stop=False, skip_group_check=True,
                        )
                        nc.tensor.matmul(
                            win, ul[:, ysl], bt[f"band2h_{si}"][:, 0],
                            start=False, stop=False, skip_group_check=True,
                        )
                        nc.tensor.matmul(
                            win, vh[:, ysl], bt[f"band2h_{si}"][:, 1],
                            start=False, stop=False, skip_group_check=True,
                        )
                        nc.tensor.matmul(
                            win, vh[:, ysl], bt[f"band2l_{si}"][:, 1],
                            start=False, stop=False, skip_group_check=True,
                        )
                        nc.tensor.matmul(
                            win, vl[:, ysl], bt[f"band2h_{si}"][:, 1],
                            start=False, stop=(si == 3), skip_group_check=True,
                        )
                    tmp = clpool.tile([128, 512], f32, tag="cl")
                    nc.vector.tensor_scalar(
                        tmp[:], ps2[:], 254.0, -1.0,
                        mybir.AluOpType.min, mybir.AluOpType.max,
                    )
                    nc.scalar.activation(ot[:, b, :, ci], tmp[:], copyf, bias=1.0)

            def store_out(n, ot):
                for b in range(n_yblk):
                    nc.sync.dma_start(
                        out_d.ap()[n, b * 128 : (b + 1) * 128], ot[:, b]
                    )

            # software pipeline: stage 2 lags stage 1 by two (n, ci) jobs
            # (uv pool bufs=3 holds the three in-flight generations), so the
            # PE never waits on the drain/clamp tail of an adjacent job.
            LAG = 2

            def run_stage2(job):
                pn, pci, puts, pvts = job
                stage2(pn, pci, puts, pvts, ots[pn])
                if pci == c - 1:
                    store_out(pn, ots.pop(pn))
                    del xt[pn]

            xt = {}
            ots = {}
            pending = []
            for n in range(n_imgs):
                for ci in range(c):
                    if ci == 0:
                        xt[n] = load_x(n)
                        if n == 0:
                            load_stage2_bands()
                        ots[n] = opool.tile(
                            [128, n_yblk, w, c], f32, tag="o", name=f"o_{n}"
                        )
                    uts, vts = stage1(n, ci, *xt[n])
                    pending.append((n, ci, uts, vts))
                    if len(pending) > LAG:
                        run_stage2(pending.pop(0))
            for job in pending:
                run_stage2(job)

    nc.compile()
    return nc


_CACHE = {}


def _get_nc():
    if "nc" not in _CACHE:
        _CACHE["nc"] = build_bass()
    return _CACHE["nc"]


def kernel(x: np.ndarray) -> np.ndarray:
    from concourse import bass_utils

    nc = _get_nc()
    x = np.ascontiguousarray(x, dtype=np.float32)
    xh = x.astype(np.float16)
    xl = (x - xh.astype(np.float32)).astype(np.float16)
    const_map = dict(make_bands())
    in_maps = [
        {
            "xh": xh[k * IMG_PER_CORE : (k + 1) * IMG_PER_CORE],
            "xl": xl[k * IMG_PER_CORE : (k + 1) * IMG_PER_CORE],
            **const_map,
        }
        for k in range(N_CORES)
    ]
    res = bass_utils.run_bass_kernel_spmd(nc, in_maps, core_ids=list(range(N_CORES)))
    _CACHE["last_result"] = res
    out = np.concatenate([r["out"] for r in res.results], axis=0)
    return out.astype(np.float32)


# revision 32
# speedup vs baseline: 2.6036x; 1.0132x over previous
"""LoG as fp16 hi/lo-pair matmuls (1 cycle/row vs fp32's 4).

Two-stage banded-matrix structure; every operand is an fp16 pair
(hi + lo ~ 21 mantissa bits, enough for the +-300k pre-clip
intermediates at the 2e-2 gate):

  x  = x_hi + x_lo           (split on host; fp16 DMA halves input bytes
                              per tensor, so total input bytes match fp32)
  G  = G_hi + G_lo           (band constants, split on host)
  u  = u_hi + u_lo           (split on-chip during the stage-1 drain:
                              u_hi = fp16(psum) via ScalarE round-on-write,
                              u_lo = psum - u_hi via VectorE)

Products kept: hi*hi + hi*lo + lo*hi (lo*lo ~ 2^-22 relative, dropped).
All three matmuls of a term target the SAME psum columns and accumulate
(start only on the first matmul of the bank's group).

Stage-2's first matmul (strip 0, u_hi x G_B-hi) streams the full 512-col
row so the accumulation group's opening matmul covers the whole psum bank
(Tile/CoreSim track pending-zero state at bank granularity).

The channel loop is software-pipelined one step: stage 2 of job k runs
after stage 1 of job k+1, so the PE fills the ~1.8 us drain-latency
bubble at each stage-1 -> stage-2 transition with independent stage-1
matmuls instead of head-of-line blocking on the last strip's drains.

Input DMAs use full-128-partition row blocks: the HWDGE descriptor
splitter serializes most of a narrower transfer onto one SDMA engine;
128-partition transfers spread across all 16 engines.
"""

import numpy as np

N_CORES = 8
BATCH = 32
IMG_PER_CORE = BATCH // N_CORES
H = W = 512
C = 3
RAD = 5
TS = 102  # chunks 0-3: 102 out rows; chunk 4 absorbs the remainder (104)


def make_chunks():
    out = []
    s = 0
    while s < H:
        n = H - s if H - s <= TS + 2 else TS
        lo = min(max(0, s - RAD), H - 128)
        out.append((s, n, lo))
        s += n
    return out


CHUNKS = make_chunks()
NCH = len(CHUNKS)
# ps1 layout = exactly 2 psum banks: chunks 0-3 hold [u 102 | v 102] at
# these slots; chunk 4's u half (104) fills bank 0's gap, its v half fills
# bank 1's gap.  (2-bank ps1 frees PSUM for ps2 bufs=4, which decouples
# stage-2 bank recycling from the drain backlog in the DVE queue.)
S1_SLOT = [0, 204, 512, 716]
S1_TAIL_U = 408  # chunk 4 u -> [408, 512)
S1_TAIL_V = 920  # chunk 4 v -> [920, 1024)
S2_WIN = [(0, 133), (123, 138), (251, 138), (379, 133)]


def make_taps():
    g = np.exp(-((np.arange(3) - 1.0) ** 2) / 2.0)
    g = g / g.sum()
    S = np.array([1, 8, 28, 56, 70, 56, 28, 8, 1], dtype=np.float64)
    D2 = np.array([1, 4, 4, -4, -10, -4, 4, 4, 1], dtype=np.float64)
    return np.convolve(g, S), np.convolve(g, D2)


def scatter_matrix(taps, n=H):
    G = np.zeros((n, n), np.float64)
    for y in range(n):
        for t in range(-RAD, RAD + 1):
            src = y + t
            if src < 0:
                src = -src
            elif src > n - 1:
                src = 2 * (n - 1) - src
            G[src, y] += taps[t + RAD]
    return G


def _pair(a):
    hi = a.astype(np.float16)
    lo = (a - hi.astype(np.float64)).astype(np.float16)
    return hi, lo


def make_bands():
    A, B = make_taps()
    GA = scatter_matrix(A)
    GB = scatter_matrix(B)
    out = {}
    # stage 1: per chunk: [A win | B win] hi and lo as separate tensors
    for i, (s, n, lo_) in enumerate(CHUNKS):
        w = np.empty((128, 2 * n), np.float64)
        w[:, 0:n] = GA[lo_ : lo_ + 128, s : s + n]
        w[:, n : 2 * n] = GB[lo_ : lo_ + 128, s : s + n]
        hi, lo = _pair(w)
        out[f"band1h_{i}"] = hi
        out[f"band1l_{i}"] = lo
    # stage 2: per strip: [128, 2, w] (plane 0 = GB for u, 1 = GA for v)
    for si in range(4):
        x0 = si * 128
        off, wd = S2_WIN[si]
        wb = np.empty((128, 2, wd), np.float64)
        wb[:, 0, :] = GB[x0 : x0 + 128, off : off + wd]
        wb[:, 1, :] = GA[x0 : x0 + 128, off : off + wd]
        for G in (GA, GB):
            assert abs(G[x0 : x0 + 128, :off]).max(initial=0) == 0
            assert abs(G[x0 : x0 + 128, off + wd :]).max(initial=0) == 0
        hi, lo = _pair(wb)
        out[f"band2h_{si}"] = hi
        out[f"band2l_{si}"] = lo
    # strip 0's u-hi matmul streams the full 512-col row (group opener)
    w = np.zeros((128, 512), np.float64)
    w[:, 0:133] = GB[0:128, 0:133]
    out["band2h_0u_full"] = _pair(w)[0]
    return out


def build_bass(n_imgs=IMG_PER_CORE, h=H, w=W, c=C):
    import concourse.bacc as bacc
    import concourse.mybir as mybir
    import concourse.tile as tile

    f32 = mybir.dt.float32
    f16 = mybir.dt.float16
    copyf = mybir.ActivationFunctionType.Copy

    nc = bacc.Bacc("TRN2", target_bir_lowering=False, debug=False)
    xh_d = nc.dram_tensor("xh", [n_imgs, h, w, c], f16, kind="ExternalInput")
    xl_d = nc.dram_tensor("xl", [n_imgs, h, w, c], f16, kind="ExternalInput")
    out_d = nc.dram_tensor("out", [n_imgs, h, w, c], f32, kind="ExternalOutput")
    bands = make_bands()
    band_d = {
        k: nc.dram_tensor(k, list(v.shape), f16, kind="ExternalInput")
        for k, v in bands.items()
    }

    n_yblk = h // 128
    tail = CHUNKS[4][1]  # 40

    with tile.TileContext(nc) as tc:
        with (
            tc.tile_pool(name="const", bufs=1) as cpool,
            tc.tile_pool(name="xin", bufs=2) as xpool,
            tc.tile_pool(name="uv", bufs=3) as uvpool,
            tc.tile_pool(name="outp", bufs=2) as opool,
            tc.tile_pool(name="clip", bufs=3) as clpool,
            tc.tile_pool(name="ps1", bufs=2, space="PSUM") as ps1pool,
            tc.tile_pool(name="ps2", bufs=4, space="PSUM") as ps2pool,
        ):
            bt = {}

            def load_band(k):
                t = cpool.tile(list(bands[k].shape), f16, name=k)
                nc.sync.dma_start(t[:], band_d[k].ap())
                bt[k] = t

            def load_x(n):
                xhr, xlr = [], []
                for i, (s, nn_, lo) in enumerate(CHUNKS):
                    th = xpool.tile([128, w, c], f16, tag=f"xh{i}", name=f"xh{i}_{n}")
                    nc.sync.dma_start(th[:], xh_d.ap()[n, lo : lo + 128])
                    xhr.append(th)
                    tl = xpool.tile([128, w, c], f16, tag=f"xl{i}", name=f"xl{i}_{n}")
                    nc.sync.dma_start(tl[:], xl_d.ap()[n, lo : lo + 128])
                    xlr.append(tl)
                    # interleave stage-1 band loads with image 0's chunks so
                    # the first matmuls aren't queued behind all 21 band
                    # transfers in the HWDGE FIFO (first-MM was at 26.4 us).
                    if n == 0:
                        load_band(f"band1h_{i}")
                        load_band(f"band1l_{i}")
                return xhr, xlr

            def load_stage2_bands():
                # needed only once stage 2 of job 0 runs (two jobs in)
                for si in range(4):
                    load_band(f"band2h_{si}")
                    load_band(f"band2l_{si}")
                load_band("band2h_0u_full")

            def stage1(n, ci, xhr, xlr):
                uts, vts = [], []
                nt = CHUNKS[4][1]
                for si in range(4):
                    ps1 = ps1pool.tile([128, 1024], f32, tag="ps1")
                    for i in range(NCH):
                        xh = xhr[i][:, si * 128 : (si + 1) * 128, ci]
                        xl = xlr[i][:, si * 128 : (si + 1) * 128, ci]
                        if i < 4:
                            # one fused [u | v] 204-col slot, 3 matmuls
                            dst = ps1[:, S1_SLOT[i] : S1_SLOT[i] + 2 * TS]
                            parts = ((dst, slice(0, 2 * TS)),)
                        else:
                            # chunk 4: u and v halves live in different banks
                            parts = (
                                (ps1[:, S1_TAIL_U : S1_TAIL_U + nt], slice(0, nt)),
                                (ps1[:, S1_TAIL_V : S1_TAIL_V + nt], slice(nt, 2 * nt)),
                            )
                        for dst, bsl in parts:
                            nc.tensor.matmul(
                                dst, xh, bt[f"band1h_{i}"][:, bsl],
                                start=True, stop=False, skip_group_check=True,
                            )
                            nc.tensor.matmul(
                                dst, xh, bt[f"band1l_{i}"][:, bsl],
                                start=False, stop=False, skip_group_check=True,
                            )
                            nc.tensor.matmul(
                                dst, xl, bt[f"band1h_{i}"][:, bsl],
                                start=False, stop=True, skip_group_check=True,
                            )
                    # drains: u_hi = fp16(psum) on ScalarE (round on write),
                    # u_lo = psum - u_hi on VectorE.
                    uh = uvpool.tile([128, h], f16, tag=f"uh{si}", name=f"uh{si}_{n}_{ci}")
                    ul = uvpool.tile([128, h], f16, tag=f"ul{si}", name=f"ul{si}_{n}_{ci}")
                    vh = uvpool.tile([128, h], f16, tag=f"vh{si}", name=f"vh{si}_{n}_{ci}")
                    vl = uvpool.tile([128, h], f16, tag=f"vl{si}", name=f"vl{si}_{n}_{ci}")
                    # [p, bank 2, chunk-pair 2, 204] view of the slot region
                    slots = (
                        ps1[:]
                        .rearrange("p (bk y) -> p bk y", bk=2)[:, :, 0 : 2 * 2 * TS]
                        .rearrange("p bk (ch x) -> p bk ch x", ch=2)
                    )
                    for hi_t, lo_t, poff, toff in (
                        (uh, ul, 0, S1_TAIL_U),
                        (vh, vl, TS, S1_TAIL_V),
                    ):
                        src = slots[:, :, :, poff : poff + TS]
                        hi4 = hi_t[:, 0 : 4 * TS].rearrange(
                            "p (bk ch x) -> p bk ch x", bk=2, ch=2
                        )
                        nc.scalar.copy(hi4, src)
                        nc.vector.tensor_tensor(
                            lo_t[:, 0 : 4 * TS].rearrange(
                                "p (bk ch x) -> p bk ch x", bk=2, ch=2
                            ),
                            src, hi4, mybir.AluOpType.subtract,
                        )
                        tsrc = ps1[:, toff : toff + tail]
                        nc.scalar.copy(hi_t[:, 4 * TS : 4 * TS + tail], tsrc)
                        nc.vector.tensor_tensor(
                            lo_t[:, 4 * TS : 4 * TS + tail],
                            tsrc, hi_t[:, 4 * TS : 4 * TS + tail],
                            mybir.AluOpType.subtract,
                        )
                    uts.append((uh, ul))
                    vts.append((vh, vl))
                return uts, vts

            def stage2(n, ci, uts, vts, ot):
                for b in range(n_yblk):
                    ps2 = ps2pool.tile([128, 512], f32, tag="ps2")
                    for si in range(4):
                        off, wd = S2_WIN[si]
                        win = ps2[:, off : off + wd]
                        ysl = slice(b * 128, (b + 1) * 128)
                        uh, ul = uts[si]
                        vh, vl = vts[si]
                        if si == 0:
                            nc.tensor.matmul(
                                ps2[:], uh[:, ysl], bt["band2h_0u_full"][:],
                                start=True, stop=False, skip_group_check=True,
                            )
                        else:
                            nc.tensor.matmul(
                                win, uh[:, ysl], bt[f"band2h_{si}"][:, 0],
                                start=False, stop=False, skip_group_check=True,
                            )
                        nc.tensor.matmul(
                            win, uh[:, ysl], bt[f"band2l_{si}"][:, 0],
                            start=False, stop=False, skip_group_check=True,
                        )
                        nc.tensor.matmul(
                            win, ul[:, ysl], bt[f"band2h_{si}"][:, 0],
                            start=False, stop=False, skip_group_check=True,
                        )
                        nc.tensor.matmul(
                            win, vh[:, ysl], bt[f"band2h_{si}"][:, 1],
                            start=False, stop=False, skip_group_check=True,
                        )
                        nc.tensor.matmul(
                            win, vh[:, ysl], bt[f"band2l_{si}"][:, 1],
                            start=False, stop=False, skip_group_check=True,
                        )
                        nc.tensor.matmul(
                            win, vl[:, ysl], bt[f"band2h_{si}"][:, 1],
                            start=False, stop=(si == 3), skip_group_check=True,
                        )
                    tmp = clpool.tile([128, 512], f32, tag="cl")
                    nc.vector.tensor_scalar(
                        tmp[:], ps2[:], 254.0, -1.0,
                        mybir.AluOpType.min, mybir.AluOpType.max,
                    )
                    nc.scalar.activation(ot[b][:, :, ci], tmp[:], copyf, bias=1.0)

            def store_out(n, ot):
                # per-block tiles: block b's DMA depends only on its own
                # three channel epilogues, so the final stores start early
                for b in range(n_yblk):
                    nc.sync.dma_start(
                        out_d.ap()[n, b * 128 : (b + 1) * 128], ot[b][:]
                    )

            # software pipeline: stage 2 lags stage 1 by two (n, ci) jobs
            # (uv pool bufs=3 holds the three in-flight generations), so the
            # PE never waits on the drain/clamp tail of an adjacent job.
            LAG = 2

            def run_stage2(job):
                pn, pci, puts, pvts = job
                stage2(pn, pci, puts, pvts, ots[pn])
                if pci == c - 1:
                    store_out(pn, ots.pop(pn))
                    del xt[pn]

            xt = {}
            ots = {}
            pending = []
            for n in range(n_imgs):
                for ci in range(c):
                    if ci == 0:
                        xt[n] = load_x(n)
                        if n == 0:
                            load_stage2_bands()
                        ots[n] = [
                            opool.tile([128, w, c], f32, tag=f"o{b}", name=f"o{b}_{n}")
                            for b in range(n_yblk)
                        ]
                    uts, vts = stage1(n, ci, *xt[n])
                    pending.append((n, ci, uts, vts))
                    if len(pending) > LAG:
                        run_stage2(pending.pop(0))
            for job in pending:
                run_stage2(job)

    nc.compile()
    return nc


_CACHE = {}


def _get_nc():
    if "nc" not in _CACHE:
        _CACHE["nc"] = build_bass()
    return _CACHE["nc"]


def kernel(x: np.ndarray) -> np.ndarray:
    from concourse import bass_utils

    nc = _get_nc()
    x = np.ascontiguousarray(x, dtype=np.float32)
    xh = x.astype(np.float16)
    xl = (x - xh.astype(np.float32)).astype(np.float16)
    const_map = dict(make_bands())
    in_maps = [
        {
            "xh": xh[k * IMG_PER_CORE : (k + 1) * IMG_PER_CORE],
            "xl": xl[k * IMG_PER_CORE : (k + 1) * IMG_PER_CORE],
            **const_map,
        }
        for k in range(N_CORES)
    ]
    res = bass_utils.run_bass_kernel_spmd(nc, in_maps, core_ids=list(range(N_CORES)))
    _CACHE["last_result"] = res
    out = np.concatenate([r["out"] for r in res.results], axis=0)
    return out.astype(np.float32)
